# revision 22
# baseline (speedup 1.0000x reference)
"""Self-contained Trainium2 Bass kernel for multi-head causal attention with RoPE.

Problem: B=2, S=2048, D=2048, H=16 heads (HD=128), fp32 reference:
    q = rope(x @ wq.T), k = rope(x @ wk.T), v = x @ wv.T
    out = softmax(q k^T / sqrt(HD) + causal_mask) @ v @ wo.T

Sharding (8 cores): core c = (b, g) with b = c // 4 (batch), g = c % 4
(head-group of 4 heads).  Each core computes its head-group's attention for
its batch and a partial output projection (columns 512g:512g+512 of the
attention output times the matching wo rows).  The host sums the 4 partial
[D, S] tensors per batch and transposes back to [S, D].

Matmul precision: the four big projections (q, k, v, wo) run as fp8e4
DoubleRow matmuls with a 3-term hi/lo split (w*x ~ wh*xh + wl*xh + wh*xl).
DoubleRow contracts two 128-partition planes per pass at half the cycle
cost, so the 3-term split costs 0.75x the bf16 rows while keeping ~11 bits
of mantissa.  x and the weights are split host-side (hi/lo planes are
interleaved in the free dim: x planes ordered (hi, lo), weight planes
(lo, hi) so one cross-term instruction pairs (wl*xh, wh*xl)).  The
attention matmuls (scores, PV) stay bf16 (contraction 128 gains nothing
from DoubleRow).

Softmax denominators come from the PE: each masked-exp tile is used as a
stationary operand against a moving ones-vector, accumulating per-q-chunk
column sums [128q, 1] in a dedicated PSUM bank (cost ~1 row per tile).
The [128, 4] reciprocal is transposed to a [1, 512] row by four tiny
SWDGE (Pool-issued) DMAs and partition-broadcast on GpSimd, then a single
DVE multiply normalizes the PV accumulator.

On-chip layout is "transposed" as in the bf16 version: Q^T/K^T as
[head_dim, seq], scoresT[k, q] feeds PV directly.  RoPE pair mixing is a
PE matmul against a signed permutation J (head dims of wq/wk permuted
host-side).  Causal masking: strictly-above-diagonal tiles skipped,
diagonal tiles at partial width with a multiplicative binary bf16 mask
after exp.
"""

import math

import numpy as np
import ml_dtypes

import concourse.bass as bass
import concourse.bacc as bacc
import concourse.mybir as mybir
from concourse.tile import TileContext
from concourse.bass_utils import run_bass_kernel_spmd
from contextlib import ExitStack

B, S, D, H = 2, 2048, 2048, 16
HD = 128          # head dim
HPG = 4           # heads per core (group)
EG = HPG * HD     # 512 head dims per core
NCORES = 8
NSTRIP = 4        # q strips per sequence
STRIP = S // NSTRIP   # 512
SKT = 128         # k tile (partition dim of scoresT)
NDT = D // 128    # 16 contraction tiles for projections
SCALE = 1.0 / math.sqrt(HD)

BF16 = mybir.dt.bfloat16
F32 = mybir.dt.float32
FP8 = mybir.dt.float8e4
DR = mybir.MatmulPerfMode.DoubleRow

# Host-side weight scaling: the reference weights have sigma ~ 1/sqrt(D) =
# 0.022, whose fp8 lo-residual (~0.0014) would underflow e4m3's subnormal
# floor (2^-9).  Scaling every weight matrix by 2^5 puts both planes in the
# normal range; q*k picks up 2^10 (cancelled in the exp scale) and v*wo
# picks up 2^10 (cancelled in the output-projection copies).
WSCALE = 32.0

LAST_EXEC_NS = None
LAST_RESULTS = None


def _build_program():
    nc = bacc.Bacc("TRN2", target_bir_lowering=False, debug=False,
                   num_devices=NCORES)
    # x / weights are pre-split into fp8 hi+lo planes host-side.
    # x planes: dim2 = (hi, lo); weight planes: dim2 = (lo, hi).
    xT_d = nc.dram_tensor("xT", [128, NDT, 2, S], FP8, kind="ExternalInput").ap()
    wqT_d = nc.dram_tensor("wqT", [128, NDT, 2, EG], FP8, kind="ExternalInput").ap()
    wkT_d = nc.dram_tensor("wkT", [128, NDT, 2, EG], FP8, kind="ExternalInput").ap()
    wvT_d = nc.dram_tensor("wvT", [128, NDT, 2, EG], FP8, kind="ExternalInput").ap()
    woT_d = nc.dram_tensor("woT", [128, HPG, 2, D], FP8, kind="ExternalInput").ap()
    cs_d = nc.dram_tensor("cs", [HD, S], BF16, kind="ExternalInput").ap()
    sn_d = nc.dram_tensor("sn", [HD, S], BF16, kind="ExternalInput").ap()
    mk_d = nc.dram_tensor("mk", [SKT, 4, STRIP], BF16, kind="ExternalInput").ap()
    outT_d = nc.dram_tensor("outT", [D, S], BF16, kind="ExternalOutput").ap()

    EXP = mybir.ActivationFunctionType.Exp

    with TileContext(nc) as tc, ExitStack() as ctx:
        wpool = ctx.enter_context(tc.tile_pool(name="wpool", bufs=1))
        kv = ctx.enter_context(tc.tile_pool(name="kv", bufs=1))
        xs = ctx.enter_context(tc.tile_pool(name="xs", bufs=2))
        qs = ctx.enter_context(tc.tile_pool(name="qs", bufs=2))
        rp = ctx.enter_context(tc.tile_pool(name="rp", bufs=2))
        ep = ctx.enter_context(tc.tile_pool(name="ep", bufs=4))
        ot = ctx.enter_context(tc.tile_pool(name="ot", bufs=3))
        po = ctx.enter_context(tc.tile_pool(name="po", bufs=6))
        nrm = ctx.enter_context(tc.tile_pool(name="nrm", bufs=2))
        scps = ctx.enter_context(tc.tile_pool(name="scps", bufs=2, space="PSUM"))
        prps = ctx.enter_context(tc.tile_pool(name="prps", bufs=2, space="PSUM"))
        pvps = ctx.enter_context(tc.tile_pool(name="pvps", bufs=2, space="PSUM"))
        wops = ctx.enter_context(tc.tile_pool(name="wops", bufs=1, space="PSUM"))
        dnps = ctx.enter_context(tc.tile_pool(name="dnps", bufs=1, space="PSUM"))

        # persistent SBUF tensors
        wq_sb = wpool.tile([128, NDT, 2, EG], FP8)
        wk_sb = wpool.tile([128, NDT, 2, EG], FP8)
        wv_sb = wpool.tile([128, NDT, 2, EG], FP8)
        wo_sb = wpool.tile([128, HPG, 2, D], FP8)
        cs_sb = wpool.tile([128, S], BF16)
        sn_sb = wpool.tile([128, S], BF16)
        mk_sb = wpool.tile([128, 4, STRIP], BF16)
        ones_sb = wpool.tile([128, 1], BF16)
        KT_sb = kv.tile([128, HPG, S], BF16)       # [e, h, sk] rope'd K^T
        V_sb = kv.tile([128, S // 128, EG], BF16)  # [sk, sk_tile, e]

        nc.vector.memset(ones_sb, 1.0)

        xt0 = xs.tile([128, NDT, 2, STRIP], FP8, tag="xt")
        x0r = xT_d[:, :, :, 0:STRIP]

        # --- DMA issue order tuned for the critical path: the first q
        # projection needs wq chunk-pairs + the first x chunk-pairs.
        for a, b in ((0, 2), (2, 4), (4, 8), (8, 12), (12, 16)):
            nc.sync.dma_start(out=wq_sb[:, a:b], in_=wqT_d[:, a:b])
            nc.sync.dma_start(out=xt0[:, a:b], in_=x0r[:, a:b])
        for c0 in range(0, NDT, 4):
            nc.sync.dma_start(out=wk_sb[:, c0:c0 + 4], in_=wkT_d[:, c0:c0 + 4])
        nc.sync.dma_start(out=wv_sb[:, 0:4], in_=wvT_d[:, 0:4])
        nc.sync.dma_start(out=cs_sb, in_=cs_d)
        nc.sync.dma_start(out=sn_sb, in_=sn_d)
        for c0 in range(4, NDT, 4):
            nc.sync.dma_start(out=wv_sb[:, c0:c0 + 4], in_=wvT_d[:, c0:c0 + 4])
        nc.sync.dma_start(out=mk_sb, in_=mk_d)
        xt1 = xs.tile([128, NDT, 2, STRIP], FP8, tag="xt")
        nc.sync.dma_start(out=xt1, in_=xT_d[:, :, :, STRIP:2 * STRIP])
        nc.sync.dma_start(out=wo_sb, in_=woT_d)

        def _proj3(out_ps, w_sb, xt, e0, ew, s0=0, sw=STRIP):
            """3-term fp8 DoubleRow accumulation: out_ps [128(e), sw] +=
            w[:, :, :, e0:e0+ew].T @ x[:, :, :, s0:s0+sw] over all 16 chunks.
            Weight planes are (lo, hi); x planes are (hi, lo)."""
            ni = NDT // 2 + NDT
            k = 0
            for t in range(NDT // 2):
                nc.tensor.matmul(out_ps,
                                 lhsT=w_sb[:, 2*t:2*t+2, 1, e0:e0+ew],
                                 rhs=xt[:, 2*t:2*t+2, 0, s0:s0+sw],
                                 start=(k == 0), stop=(k == ni - 1),
                                 perf_mode=DR)
                k += 1
            for c in range(NDT):
                nc.tensor.matmul(out_ps,
                                 lhsT=w_sb[:, c, :, e0:e0+ew],
                                 rhs=xt[:, c, :, s0:s0+sw],
                                 start=(k == 0), stop=(k == ni - 1),
                                 perf_mode=DR)
                k += 1

        def _projv3(out_ps, xt, st):
            """V projection: stationary = x chunk [128, 2, 128(s)],
            moving = wv planes [128, 2, 512(e)]."""
            ni = NDT // 2 + NDT
            k = 0
            sl = slice(st * 128, (st + 1) * 128)
            for t in range(NDT // 2):
                nc.tensor.matmul(out_ps,
                                 lhsT=xt[:, 2*t:2*t+2, 0, sl],
                                 rhs=wv_sb[:, 2*t:2*t+2, 1, :],
                                 start=(k == 0), stop=(k == ni - 1),
                                 perf_mode=DR)
                k += 1
            for c in range(NDT):
                nc.tensor.matmul(out_ps,
                                 lhsT=xt[:, c, :, sl],
                                 rhs=wv_sb[:, c, :, :],
                                 start=(k == 0), stop=(k == ni - 1),
                                 perf_mode=DR)
                k += 1

        deferred = []
        COPY = mybir.ActivationFunctionType.Copy
        UNSCALE = 1.0 / (WSCALE * WSCALE)

        def _emit_wo_tile(j, otile, nt, borrow=False):
            """One partial-output-projection tile: 3-term fp8 DR over the
            4 head chunks.  otile: [128(e), HPG, 2(hi,lo), STRIP] fp8.
            The copies also undo the host-side 2^10 weight scaling."""
            s0 = j * STRIP
            nsl = slice(nt * 128, (nt + 1) * 128)
            if borrow:
                # attention is over: rotate across every psum pool so the
                # tail pipelines instead of serialising on one bank
                r = nt % 4
                if r == 0:
                    pr = wops.tile([128, STRIP], F32, tag="pr")
                elif r == 1:
                    pr = scps.tile([128, STRIP], F32, tag="sc")
                elif r == 2:
                    pr = prps.tile([128, STRIP], F32, tag="mm")
                else:
                    pr = pvps.tile([128, STRIP], F32, tag="pv")
            else:
                pr = wops.tile([128, STRIP], F32, tag="pr")
            k = 0
            for t in range(HPG // 2):
                nc.tensor.matmul(pr,
                                 lhsT=wo_sb[:, 2*t:2*t+2, 1, nsl],
                                 rhs=otile[:, 2*t:2*t+2, 0, :],
                                 start=(k == 0), stop=False,
                                 perf_mode=DR)
                k += 1
            for h in range(HPG):
                nc.tensor.matmul(pr,
                                 lhsT=wo_sb[:, h, :, nsl],
                                 rhs=otile[:, h, :, :],
                                 start=False, stop=(h == HPG - 1),
                                 perf_mode=DR)
            pr_sb = po.tile([128, STRIP], BF16, tag="po")
            if nt % 2 == 1:
                nc.scalar.activation(pr_sb, pr, COPY, scale=UNSCALE)
            else:
                nc.vector.tensor_scalar_mul(pr_sb, pr, UNSCALE)
            nc.sync.dma_start(
                out=outT_d[nt * 128:(nt + 1) * 128, s0:s0 + STRIP], in_=pr_sb)

        for j in range(NSTRIP):
            s0 = j * STRIP
            if j == 0:
                xt = xt0
            elif j == 1:
                xt = xt1
            else:
                xt = xs.tile([128, NDT, 2, STRIP], FP8, tag="xt")
                nc.sync.dma_start(out=xt, in_=xT_d[:, :, :, s0:s0 + STRIP])
            qt = qs.tile([128, HPG, STRIP], BF16, tag="qt")

            # --- projections + RoPE for this strip ---
            if j == 0:
                # strip 0: chunk-outer over the 2-chunk DMA pairs so the first
                # matmuls start as soon as (wq pair 0, x pair 0) land; the
                # 4 concurrent head accumulators borrow the idle psum bufs
                qp0 = prps.tile([128, STRIP], F32, tag="mm")
                qp1 = prps.tile([128, STRIP], F32, tag="mm")
                qp2 = scps.tile([128, STRIP], F32, tag="sc")
                qp3 = scps.tile([128, STRIP], F32, tag="sc")
                qps0 = [qp0, qp1, qp2, qp3]
                ni = NDT // 2 + NDT
                kk = [0] * HPG
                for t in range(NDT // 2):
                    for h in range(HPG):
                        e0 = h * HD
                        nc.tensor.matmul(qps0[h],
                                         lhsT=wq_sb[:, 2*t:2*t+2, 1, e0:e0+HD],
                                         rhs=xt[:, 2*t:2*t+2, 0, :],
                                         start=(kk[h] == 0), stop=False,
                                         perf_mode=DR)
                        kk[h] += 1
                        for c in (2*t, 2*t+1):
                            nc.tensor.matmul(qps0[h],
                                             lhsT=wq_sb[:, c, :, e0:e0+HD],
                                             rhs=xt[:, c, :, :],
                                             start=False, stop=(kk[h] == ni - 1),
                                             perf_mode=DR)
                            kk[h] += 1
            # Projections run two "stages" ahead of the J/RoPE mix: the J
            # matmul for head h-1 is emitted after head h's Q/K accumulation,
            # so the PE never head-of-line blocks on the ACT psum->sbuf copy.
            qk_sb = {}

            def _jrope(h):
                # pair-swap via two SBUF->SBUF DMAs (the sign of the rotated
                # half is folded into the sn table host-side)
                for which, dst in (("q", qt[:, h, :]),
                                   ("k", KT_sb[:, h, s0:s0 + STRIP])):
                    src = qk_sb.pop((h, which))
                    j_sb = rp.tile([128, STRIP], BF16, tag="jsb")
                    nc.gpsimd.dma_start(out=j_sb[0:64, :], in_=src[64:128, :])
                    nc.gpsimd.dma_start(out=j_sb[64:128, :], in_=src[0:64, :])
                    t1 = rp.tile([128, STRIP], BF16, tag="ra")
                    nc.vector.tensor_mul(t1, src, cs_sb[:, s0:s0 + STRIP])
                    t2 = rp.tile([128, STRIP], BF16, tag="rb")
                    nc.vector.tensor_mul(t2, j_sb, sn_sb[:, s0:s0 + STRIP])
                    nc.vector.tensor_add(dst, t1, t2)

            for h in range(HPG):
                e0 = h * HD
                if j == 0:
                    q_ps = qps0[h]
                else:
                    q_ps = prps.tile([128, STRIP], F32, tag="mm")
                    _proj3(q_ps, wq_sb, xt, e0, HD)
                q_sb = rp.tile([128, STRIP], BF16, tag="qsb", bufs=4)
                nc.scalar.copy(q_sb, q_ps)
                qk_sb[(h, "q")] = q_sb

                k_ps = prps.tile([128, STRIP], F32, tag="mm")
                _proj3(k_ps, wk_sb, xt, e0, HD)
                k_sb = rp.tile([128, STRIP], BF16, tag="ksb", bufs=4)
                nc.vector.tensor_copy(k_sb, k_ps)
                qk_sb[(h, "k")] = k_sb
                if h > 0:
                    _jrope(h - 1)

            for st in range(4):
                v_ps = prps.tile([128, EG], F32, tag="mm")
                _projv3(v_ps, xt, st)
                nc.vector.tensor_copy(V_sb[:, j * 4 + st, :], v_ps)
                if st == 0:
                    _jrope(HPG - 1)

            # --- attention for this strip ---
            otile = ot.tile([128, HPG, 2, STRIP], FP8, tag="ot")
            nsk = 4 * j + 4
            for h in range(HPG):
                # previous strip's deferred wo tiles, 4 per head boundary
                for _ in range(4):
                    if deferred:
                        _emit_wo_tile(*deferred.pop(0))
                e0 = h * HD
                pv_ps = pvps.tile([128, STRIP], F32, tag="pv")
                den_ps = dnps.tile([128, 4], F32, tag="dn")
                rcb_c = nrm.tile([128, 4], F32, tag="rcbc")
                rcb_row = nrm.tile([1, STRIP], F32, tag="rcbr")
                for skt in range(nsk):
                    d = skt - 4 * j   # >= 0 on the diagonal block
                    w = STRIP - 128 * d if d > 0 else STRIP
                    dd = max(d, 0)
                    first = (skt == 0)
                    sc_ps = scps.tile([128, STRIP], F32, tag="sc")
                    nc.tensor.matmul(sc_ps[:, 0:w],
                                     lhsT=KT_sb[:, h, skt * 128:(skt + 1) * 128],
                                     rhs=qt[:, h, STRIP - w:STRIP],
                                     start=True, stop=True)
                    ex = ep.tile([128, STRIP], BF16, tag="ex")
                    nc.scalar.activation(ex[:, 0:w], sc_ps[:, 0:w], EXP,
                                         scale=SCALE / (WSCALE * WSCALE))
                    if d >= 0:
                        exm = ep.tile([128, STRIP], BF16, tag="exm")
                        nc.vector.tensor_mul(exm[:, 0:w], ex[:, 0:w],
                                             mk_sb[:, d, STRIP - w:STRIP])
                    else:
                        exm = ex
                    nc.tensor.matmul(pv_ps[:, STRIP - w:STRIP],
                                     lhsT=V_sb[:, skt, e0:e0 + HD],
                                     rhs=exm[:, 0:w], start=first,
                                     stop=(skt == nsk - 1))
                    # denominator columns: exm chunk as stationary x ones.
                    # NOTE: start=True zeroes the whole PSUM bank, so only the
                    # very first den matmul of the head-strip starts the bank;
                    # every column then accumulates into the zeroed bank.
                    for qc in range(dd, 4):
                        last = (skt == nsk - 1 - (3 - qc))
                        nc.tensor.matmul(
                            den_ps[:, qc:qc + 1],
                            lhsT=exm[:, (qc - dd) * 128:(qc - dd + 1) * 128],
                            rhs=ones_sb,
                            start=(first and qc == 0),
                            stop=last,
                            skip_group_check=True)
                        if last:
                            nc.vector.reciprocal(rcb_c[:, qc:qc + 1],
                                                 den_ps[:, qc:qc + 1])
                            nc.gpsimd.dma_start(
                                out=rcb_row[:, qc * 128:(qc + 1) * 128],
                                in_=rcb_c[:, qc:qc + 1])
                # free the pv psum bank quickly: copy the unnormalised
                # accumulator to SBUF; the rest of the normalisation pipeline
                # runs off the PE critical path (its deadline is the deferred
                # wo of this strip, a full strip away).
                pv_sb = rp.tile([128, STRIP], BF16, tag="pvs")
                nc.vector.tensor_copy(pv_sb, pv_ps)
                rcb_bc = nrm.tile([128, STRIP], F32, tag="rcbb")
                nc.gpsimd.partition_broadcast(rcb_bc, rcb_row)
                m_sb = rp.tile([128, STRIP], BF16, tag="msb")
                nc.vector.tensor_mul(m_sb, pv_sb, rcb_bc)
                nc.scalar.copy(otile[:, h, 0, :], m_sb)
                nc.vector.tensor_sub(otile[:, h, 1, :], m_sb, otile[:, h, 0, :])

            while j == NSTRIP - 1 and deferred:
                _emit_wo_tile(*deferred.pop(0))
            # wo for this strip is deferred into the next strip's attention
            # window (PE-light there); strip 3 emits inline above.
            if j < NSTRIP - 1:
                deferred.extend((j, otile, nt) for nt in range(NDT))
            else:
                for nt in range(NDT):
                    _emit_wo_tile(j, otile, nt, borrow=True)

    return nc


_PERM = np.concatenate([np.arange(0, HD, 2), np.arange(1, HD, 2)])


def _split_fp8(a):
    """Split f32 array into (hi, lo) fp8e4 planes with hi + lo ~ a."""
    fp8 = ml_dtypes.float8_e4m3
    hi = a.astype(fp8)
    lo = (a - hi.astype(np.float32)).astype(fp8)
    return hi, lo


def _host_prep(x, wq, wk, wv, wo, freqs_cos, freqs_sin, mask):
    bf16 = ml_dtypes.bfloat16
    x = np.asarray(x, np.float32)
    wq = np.asarray(wq, np.float32)
    wk = np.asarray(wk, np.float32)
    wv = np.asarray(wv, np.float32)
    wo = np.asarray(wo, np.float32)
    cos = np.asarray(freqs_cos, np.float32)   # [S, HD/2]
    sin = np.asarray(freqs_sin, np.float32)
    mask = np.asarray(mask, np.float32)

    cosH = cos.T                               # [64, S]
    sinH = sin.T
    cs = np.vstack([cosH, cosH]).astype(bf16)  # [128, S]
    # rows 0:64 multiply the swapped-in q[64:128] and need the J sign flip
    sn = np.vstack([-sinH, sinH]).astype(bf16)

    # multiplicative binary causal mask for the 4 diagonal-tile flavours:
    # mk[k, d, q] = 1 where allowed, 0 where masked
    mk = np.empty((SKT, 4, STRIP), np.float32)
    for d_ in range(4):
        sub = mask[0:STRIP, d_ * SKT:(d_ + 1) * SKT]   # [q, k]
        mk[:, d_, :] = np.where(np.isfinite(sub), 1.0, 0.0).T
    mk = mk.astype(bf16)

    perm_g = np.concatenate([h * HD + _PERM for h in range(HPG)])

    def _pack_w(wT):
        # wT: [D, EG] f32 -> [128, NDT, 2(lo,hi), EG] fp8, scaled by WSCALE
        hi, lo = _split_fp8(wT * WSCALE)
        st = np.stack([lo, hi], axis=1)            # [D, 2, EG]
        return np.ascontiguousarray(
            st.reshape(NDT, 128, 2, EG).transpose(1, 0, 2, 3))

    def _pack_x(xT):
        # xT: [D, S] f32 -> [128, NDT, 2(hi,lo), S] fp8
        hi, lo = _split_fp8(xT)
        st = np.stack([hi, lo], axis=1)            # [D, 2, S]
        return np.ascontiguousarray(
            st.reshape(NDT, 128, 2, S).transpose(1, 0, 2, 3))

    in_maps = []
    for c in range(NCORES):
        b, g = c // HPG, c % HPG
        rows = slice(g * EG, (g + 1) * EG)
        wq_g = wq[rows][perm_g]                # [EG, D], head dims permuted
        wk_g = wk[rows][perm_g]
        wv_g = wv[rows]
        wo_g = wo[:, rows]                     # [D, EG]
        # woT: [EG, D] -> [128, HPG(h-chunk), 2(lo,hi), D]
        hi, lo = _split_fp8(wo_g.T * WSCALE)
        wo_st = np.stack([lo, hi], axis=1)     # [EG, 2, D]
        wo_pk = np.ascontiguousarray(
            wo_st.reshape(HPG, 128, 2, D).transpose(1, 0, 2, 3))
        in_maps.append({
            "xT": _pack_x(x[b].T),
            "wqT": _pack_w(wq_g.T),
            "wkT": _pack_w(wk_g.T),
            "wvT": _pack_w(wv_g.T),
            "woT": wo_pk,
            "cs": cs, "sn": sn, "mk": mk,
        })
    return in_maps


def kernel(x, wq, wk, wv, wo, freqs_cos, freqs_sin, mask, start_pos):
    global LAST_EXEC_NS, LAST_RESULTS
    in_maps = _host_prep(x, wq, wk, wv, wo, freqs_cos, freqs_sin, mask)
    nc = _build_program()
    nc.finalize()
    res = run_bass_kernel_spmd(nc, in_maps, core_ids=list(range(NCORES)),
                               trace=False)
    LAST_EXEC_NS = res.exec_time_ns
    LAST_RESULTS = res
    out = np.empty((B, S, D), np.float32)
    for b in range(B):
        acc = np.zeros((D, S), np.float32)
        for g in range(HPG):
            acc += res.results[b * HPG + g]["outT"].astype(np.float32)
        out[b] = acc.T
    return out


# revision 23
# speedup vs baseline: 1.0831x; 1.0831x over previous
"""Self-contained Trainium2 Bass kernel for multi-head causal attention with RoPE.

Problem: B=2, S=2048, D=2048, H=16 heads (HD=128), fp32 reference:
    q = rope(x @ wq.T), k = rope(x @ wk.T), v = x @ wv.T
    out = softmax(q k^T / sqrt(HD) + causal_mask) @ v @ wo.T

Sharding (8 cores): core c = (b, g) with b = c // 4 (batch), g = c % 4
(head-group of 4 heads).  Each core computes its head-group's attention for
its batch and a partial output projection (columns 512g:512g+512 of the
attention output times the matching wo rows).  The host sums the 4 partial
[D, S] tensors per batch and transposes back to [S, D].

Matmul precision: the four big projections (q, k, v, wo) run as fp8e4
DoubleRow matmuls with a 3-term hi/lo split (w*x ~ wh*xh + wl*xh + wh*xl).
DoubleRow contracts two 128-partition planes per pass at half the cycle
cost, so the 3-term split costs 0.75x the bf16 rows while keeping ~11 bits
of mantissa.  x and the weights are split host-side (hi/lo planes are
interleaved in the free dim: x planes ordered (hi, lo), weight planes
(lo, hi) so one cross-term instruction pairs (wl*xh, wh*xl)).  The
attention matmuls (scores, PV) stay bf16 (contraction 128 gains nothing
from DoubleRow).

Softmax denominators come from the PE: each masked-exp tile is used as a
stationary operand against a moving ones-vector, accumulating per-q-chunk
column sums [128q, 1] in a dedicated PSUM bank (cost ~1 row per tile).
The [128, 4] reciprocal is transposed to a [1, 512] row by four tiny
SWDGE (Pool-issued) DMAs and partition-broadcast on GpSimd, then a single
DVE multiply normalizes the PV accumulator.

On-chip layout is "transposed" as in the bf16 version: Q^T/K^T as
[head_dim, seq], scoresT[k, q] feeds PV directly.  RoPE pair mixing is a
PE matmul against a signed permutation J (head dims of wq/wk permuted
host-side).  Causal masking: strictly-above-diagonal tiles skipped,
diagonal tiles at partial width with a multiplicative binary bf16 mask
after exp.
"""

import math

import numpy as np
import ml_dtypes

import concourse.bass as bass
import concourse.bacc as bacc
import concourse.mybir as mybir
from concourse.tile import TileContext
from concourse.bass_utils import run_bass_kernel_spmd
from contextlib import ExitStack

B, S, D, H = 2, 2048, 2048, 16
HD = 128          # head dim
HPG = 4           # heads per core (group)
EG = HPG * HD     # 512 head dims per core
NCORES = 8
NSTRIP = 4        # q strips per sequence
STRIP = S // NSTRIP   # 512
SKT = 128         # k tile (partition dim of scoresT)
NDT = D // 128    # 16 contraction tiles for projections
SCALE = 1.0 / math.sqrt(HD)

BF16 = mybir.dt.bfloat16
F32 = mybir.dt.float32
FP8 = mybir.dt.float8e4
DR = mybir.MatmulPerfMode.DoubleRow

# Host-side weight scaling: the reference weights have sigma ~ 1/sqrt(D) =
# 0.022, whose fp8 lo-residual (~0.0014) would underflow e4m3's subnormal
# floor (2^-9).  Scaling every weight matrix by 2^5 puts both planes in the
# normal range; q*k picks up 2^10 (cancelled in the exp scale) and v*wo
# picks up 2^10 (cancelled in the output-projection copies).
WSCALE = 32.0

LAST_EXEC_NS = None
LAST_RESULTS = None


def _build_program():
    nc = bacc.Bacc("TRN2", target_bir_lowering=False, debug=False,
                   num_devices=NCORES)
    # x / weights are pre-split into fp8 hi+lo planes host-side.
    # x planes: dim2 = (hi, lo); weight planes: dim2 = (lo, hi).
    xT_d = nc.dram_tensor("xT", [128, NDT, 2, S], FP8, kind="ExternalInput").ap()
    wqT_d = nc.dram_tensor("wqT", [128, NDT, 2, EG], FP8, kind="ExternalInput").ap()
    wkT_d = nc.dram_tensor("wkT", [128, NDT, 2, EG], FP8, kind="ExternalInput").ap()
    wvT_d = nc.dram_tensor("wvT", [128, NDT, 2, EG], FP8, kind="ExternalInput").ap()
    woT_d = nc.dram_tensor("woT", [128, HPG, 2, D], FP8, kind="ExternalInput").ap()
    cs_d = nc.dram_tensor("cs", [HD, S], BF16, kind="ExternalInput").ap()
    sn_d = nc.dram_tensor("sn", [HD, S], BF16, kind="ExternalInput").ap()
    mk_d = nc.dram_tensor("mk", [SKT, 4, STRIP], BF16, kind="ExternalInput").ap()
    jt_d = nc.dram_tensor("jt", [HD, HD], BF16, kind="ExternalInput").ap()
    outT_d = nc.dram_tensor("outT", [D, S], BF16, kind="ExternalOutput").ap()

    EXP = mybir.ActivationFunctionType.Exp

    with TileContext(nc) as tc, ExitStack() as ctx:
        wpool = ctx.enter_context(tc.tile_pool(name="wpool", bufs=1))
        kv = ctx.enter_context(tc.tile_pool(name="kv", bufs=1))
        xs = ctx.enter_context(tc.tile_pool(name="xs", bufs=2))
        qs = ctx.enter_context(tc.tile_pool(name="qs", bufs=2))
        rp = ctx.enter_context(tc.tile_pool(name="rp", bufs=2))
        ep = ctx.enter_context(tc.tile_pool(name="ep", bufs=4))
        ot = ctx.enter_context(tc.tile_pool(name="ot", bufs=3))
        po = ctx.enter_context(tc.tile_pool(name="po", bufs=6))
        nrm = ctx.enter_context(tc.tile_pool(name="nrm", bufs=2))
        scps = ctx.enter_context(tc.tile_pool(name="scps", bufs=2, space="PSUM"))
        prps = ctx.enter_context(tc.tile_pool(name="prps", bufs=2, space="PSUM"))
        pvps = ctx.enter_context(tc.tile_pool(name="pvps", bufs=2, space="PSUM"))
        wops = ctx.enter_context(tc.tile_pool(name="wops", bufs=1, space="PSUM"))
        dnps = ctx.enter_context(tc.tile_pool(name="dnps", bufs=1, space="PSUM"))

        # persistent SBUF tensors
        wq_sb = wpool.tile([128, NDT, 2, EG], FP8)
        wk_sb = wpool.tile([128, NDT, 2, EG], FP8)
        wv_sb = wpool.tile([128, NDT, 2, EG], FP8)
        wo_sb = wpool.tile([128, HPG, 2, D], FP8)
        cs_sb = wpool.tile([128, S], BF16)
        sn_sb = wpool.tile([128, S], BF16)
        mk_sb = wpool.tile([128, 4, STRIP], BF16)
        jt_sb = wpool.tile([HD, HD], BF16)
        ones_sb = wpool.tile([128, 1], BF16)
        KT_sb = kv.tile([128, HPG, S], BF16)       # [e, h, sk] rope'd K^T
        V_sb = kv.tile([128, S // 128, EG], BF16)  # [sk, sk_tile, e]

        nc.vector.memset(ones_sb, 1.0)

        xt0 = xs.tile([128, NDT, 2, STRIP], FP8, tag="xt")
        x0r = xT_d[:, :, :, 0:STRIP]

        # --- DMA issue order tuned for the critical path: the first q
        # projection needs wq chunk-pairs + the first x chunk-pairs.
        for a, b in ((0, 2), (2, 4), (4, 8), (8, 12), (12, 16)):
            nc.sync.dma_start(out=wq_sb[:, a:b], in_=wqT_d[:, a:b])
            nc.sync.dma_start(out=xt0[:, a:b], in_=x0r[:, a:b])
        for c0 in range(0, NDT, 4):
            nc.sync.dma_start(out=wk_sb[:, c0:c0 + 4], in_=wkT_d[:, c0:c0 + 4])
        nc.sync.dma_start(out=wv_sb[:, 0:4], in_=wvT_d[:, 0:4])
        nc.sync.dma_start(out=cs_sb, in_=cs_d)
        nc.sync.dma_start(out=sn_sb, in_=sn_d)
        for c0 in range(4, NDT, 4):
            nc.sync.dma_start(out=wv_sb[:, c0:c0 + 4], in_=wvT_d[:, c0:c0 + 4])
        nc.sync.dma_start(out=mk_sb, in_=mk_d)
        nc.sync.dma_start(out=jt_sb, in_=jt_d)
        xt1 = xs.tile([128, NDT, 2, STRIP], FP8, tag="xt")
        nc.sync.dma_start(out=xt1, in_=xT_d[:, :, :, STRIP:2 * STRIP])
        nc.sync.dma_start(out=wo_sb, in_=woT_d)

        def _proj3(out_ps, w_sb, xt, e0, ew, s0=0, sw=STRIP):
            """3-term fp8 DoubleRow accumulation: out_ps [128(e), sw] +=
            w[:, :, :, e0:e0+ew].T @ x[:, :, :, s0:s0+sw] over all 16 chunks.
            Weight planes are (lo, hi); x planes are (hi, lo)."""
            ni = NDT // 2 + NDT
            k = 0
            for t in range(NDT // 2):
                nc.tensor.matmul(out_ps,
                                 lhsT=w_sb[:, 2*t:2*t+2, 1, e0:e0+ew],
                                 rhs=xt[:, 2*t:2*t+2, 0, s0:s0+sw],
                                 start=(k == 0), stop=(k == ni - 1),
                                 perf_mode=DR)
                k += 1
            for c in range(NDT):
                nc.tensor.matmul(out_ps,
                                 lhsT=w_sb[:, c, :, e0:e0+ew],
                                 rhs=xt[:, c, :, s0:s0+sw],
                                 start=(k == 0), stop=(k == ni - 1),
                                 perf_mode=DR)
                k += 1

        def _projv3(out_ps, xt, st):
            """V projection: stationary = x chunk [128, 2, 128(s)],
            moving = wv planes [128, 2, 512(e)]."""
            ni = NDT // 2 + NDT
            k = 0
            sl = slice(st * 128, (st + 1) * 128)
            for t in range(NDT // 2):
                nc.tensor.matmul(out_ps,
                                 lhsT=xt[:, 2*t:2*t+2, 0, sl],
                                 rhs=wv_sb[:, 2*t:2*t+2, 1, :],
                                 start=(k == 0), stop=(k == ni - 1),
                                 perf_mode=DR)
                k += 1
            for c in range(NDT):
                nc.tensor.matmul(out_ps,
                                 lhsT=xt[:, c, :, sl],
                                 rhs=wv_sb[:, c, :, :],
                                 start=(k == 0), stop=(k == ni - 1),
                                 perf_mode=DR)
                k += 1

        deferred = []
        COPY = mybir.ActivationFunctionType.Copy
        UNSCALE = 1.0 / (WSCALE * WSCALE)

        def _emit_wo_tile(j, otile, nt, borrow=False):
            """One partial-output-projection tile: 3-term fp8 DR over the
            4 head chunks.  otile: [128(e), HPG, 2(hi,lo), STRIP] fp8.
            The copies also undo the host-side 2^10 weight scaling."""
            s0 = j * STRIP
            nsl = slice(nt * 128, (nt + 1) * 128)
            if borrow:
                # attention is over: rotate across every psum pool so the
                # tail pipelines instead of serialising on one bank
                r = nt % 4
                if r == 0:
                    pr = wops.tile([128, STRIP], F32, tag="pr")
                elif r == 1:
                    pr = scps.tile([128, STRIP], F32, tag="sc")
                elif r == 2:
                    pr = prps.tile([128, STRIP], F32, tag="mm")
                else:
                    pr = pvps.tile([128, STRIP], F32, tag="pv")
            else:
                pr = wops.tile([128, STRIP], F32, tag="pr")
            k = 0
            for t in range(HPG // 2):
                nc.tensor.matmul(pr,
                                 lhsT=wo_sb[:, 2*t:2*t+2, 1, nsl],
                                 rhs=otile[:, 2*t:2*t+2, 0, :],
                                 start=(k == 0), stop=False,
                                 perf_mode=DR)
                k += 1
            for h in range(HPG):
                nc.tensor.matmul(pr,
                                 lhsT=wo_sb[:, h, :, nsl],
                                 rhs=otile[:, h, :, :],
                                 start=False, stop=(h == HPG - 1),
                                 perf_mode=DR)
            pr_sb = po.tile([128, STRIP], BF16, tag="po")
            if nt % 2 == 1:
                nc.scalar.activation(pr_sb, pr, COPY, scale=UNSCALE)
            else:
                nc.vector.tensor_scalar_mul(pr_sb, pr, UNSCALE)
            nc.sync.dma_start(
                out=outT_d[nt * 128:(nt + 1) * 128, s0:s0 + STRIP], in_=pr_sb)

        for j in range(NSTRIP):
            s0 = j * STRIP
            if j == 0:
                xt = xt0
            elif j == 1:
                xt = xt1
            else:
                xt = xs.tile([128, NDT, 2, STRIP], FP8, tag="xt")
                nc.sync.dma_start(out=xt, in_=xT_d[:, :, :, s0:s0 + STRIP])
            qt = qs.tile([128, HPG, STRIP], BF16, tag="qt")

            # --- projections + RoPE for this strip ---
            if j == 0:
                # strip 0: chunk-outer over the 2-chunk DMA pairs so the first
                # matmuls start as soon as (wq pair 0, x pair 0) land; the
                # 4 concurrent head accumulators borrow the idle psum bufs
                qp0 = prps.tile([128, STRIP], F32, tag="mm")
                qp1 = prps.tile([128, STRIP], F32, tag="mm")
                qp2 = scps.tile([128, STRIP], F32, tag="sc")
                qp3 = scps.tile([128, STRIP], F32, tag="sc")
                qps0 = [qp0, qp1, qp2, qp3]
                ni = NDT // 2 + NDT
                kk = [0] * HPG
                for t in range(NDT // 2):
                    for h in range(HPG):
                        e0 = h * HD
                        nc.tensor.matmul(qps0[h],
                                         lhsT=wq_sb[:, 2*t:2*t+2, 1, e0:e0+HD],
                                         rhs=xt[:, 2*t:2*t+2, 0, :],
                                         start=(kk[h] == 0), stop=False,
                                         perf_mode=DR)
                        kk[h] += 1
                        for c in (2*t, 2*t+1):
                            nc.tensor.matmul(qps0[h],
                                             lhsT=wq_sb[:, c, :, e0:e0+HD],
                                             rhs=xt[:, c, :, :],
                                             start=False, stop=(kk[h] == ni - 1),
                                             perf_mode=DR)
                            kk[h] += 1
            # Projections run two "stages" ahead of the J/RoPE mix: the J
            # matmul for head h-1 is emitted after head h's Q/K accumulation,
            # so the PE never head-of-line blocks on the ACT psum->sbuf copy.
            qk_sb = {}

            def _jrope(h):
                for which, dst in (("q", qt[:, h, :]),
                                   ("k", KT_sb[:, h, s0:s0 + STRIP])):
                    src = qk_sb.pop((h, which))
                    if j == 0:
                        jps = pvps.tile([128, STRIP], F32, tag="pv")
                    else:
                        jps = scps.tile([128, STRIP], F32, tag="sc")
                    nc.tensor.matmul(jps, lhsT=jt_sb, rhs=src,
                                     start=True, stop=True)
                    j_sb = rp.tile([128, STRIP], BF16, tag="jsb")
                    nc.scalar.copy(j_sb, jps)
                    t1 = rp.tile([128, STRIP], BF16, tag="ra")
                    nc.vector.tensor_mul(t1, src, cs_sb[:, s0:s0 + STRIP])
                    t2 = rp.tile([128, STRIP], BF16, tag="rb")
                    nc.vector.tensor_mul(t2, j_sb, sn_sb[:, s0:s0 + STRIP])
                    nc.vector.tensor_add(dst, t1, t2)

            for h in range(HPG):
                e0 = h * HD
                if j == 0:
                    q_ps = qps0[h]
                else:
                    q_ps = prps.tile([128, STRIP], F32, tag="mm")
                    _proj3(q_ps, wq_sb, xt, e0, HD)
                q_sb = rp.tile([128, STRIP], BF16, tag="qsb", bufs=4)
                nc.scalar.copy(q_sb, q_ps)
                qk_sb[(h, "q")] = q_sb

                k_ps = prps.tile([128, STRIP], F32, tag="mm")
                _proj3(k_ps, wk_sb, xt, e0, HD)
                k_sb = rp.tile([128, STRIP], BF16, tag="ksb", bufs=4)
                nc.vector.tensor_copy(k_sb, k_ps)
                qk_sb[(h, "k")] = k_sb
                if h > 0:
                    _jrope(h - 1)

            for st in range(4):
                v_ps = prps.tile([128, EG], F32, tag="mm")
                _projv3(v_ps, xt, st)
                nc.vector.tensor_copy(V_sb[:, j * 4 + st, :], v_ps)
                if st == 0:
                    _jrope(HPG - 1)

            # --- attention for this strip ---
            otile = ot.tile([128, HPG, 2, STRIP], FP8, tag="ot")
            nsk = 4 * j + 4
            for h in range(HPG):
                # previous strip's deferred wo tiles, 4 per head boundary
                for _ in range(4):
                    if deferred:
                        _emit_wo_tile(*deferred.pop(0))
                e0 = h * HD
                pv_ps = pvps.tile([128, STRIP], F32, tag="pv")
                den_ps = dnps.tile([128, 4], F32, tag="dn")
                rcb_c = nrm.tile([128, 4], F32, tag="rcbc")
                rcb_row = nrm.tile([1, STRIP], F32, tag="rcbr")
                for skt in range(nsk):
                    d = skt - 4 * j   # >= 0 on the diagonal block
                    w = STRIP - 128 * d if d > 0 else STRIP
                    dd = max(d, 0)
                    first = (skt == 0)
                    sc_ps = scps.tile([128, STRIP], F32, tag="sc")
                    nc.tensor.matmul(sc_ps[:, 0:w],
                                     lhsT=KT_sb[:, h, skt * 128:(skt + 1) * 128],
                                     rhs=qt[:, h, STRIP - w:STRIP],
                                     start=True, stop=True)
                    ex = ep.tile([128, STRIP], BF16, tag="ex")
                    nc.scalar.activation(ex[:, 0:w], sc_ps[:, 0:w], EXP,
                                         scale=SCALE / (WSCALE * WSCALE))
                    if d >= 0:
                        exm = ep.tile([128, STRIP], BF16, tag="exm")
                        nc.vector.tensor_mul(exm[:, 0:w], ex[:, 0:w],
                                             mk_sb[:, d, STRIP - w:STRIP])
                    else:
                        exm = ex
                    nc.tensor.matmul(pv_ps[:, STRIP - w:STRIP],
                                     lhsT=V_sb[:, skt, e0:e0 + HD],
                                     rhs=exm[:, 0:w], start=first,
                                     stop=(skt == nsk - 1))
                    # denominator columns: exm chunk as stationary x ones.
                    # NOTE: start=True zeroes the whole PSUM bank, so only the
                    # very first den matmul of the head-strip starts the bank;
                    # every column then accumulates into the zeroed bank.
                    for qc in range(dd, 4):
                        last = (skt == nsk - 1 - (3 - qc))
                        nc.tensor.matmul(
                            den_ps[:, qc:qc + 1],
                            lhsT=exm[:, (qc - dd) * 128:(qc - dd + 1) * 128],
                            rhs=ones_sb,
                            start=(first and qc == 0),
                            stop=last,
                            skip_group_check=True)
                        if last:
                            nc.vector.reciprocal(rcb_c[:, qc:qc + 1],
                                                 den_ps[:, qc:qc + 1])
                            nc.gpsimd.dma_start(
                                out=rcb_row[:, qc * 128:(qc + 1) * 128],
                                in_=rcb_c[:, qc:qc + 1])
                # free the pv psum bank quickly: copy the unnormalised
                # accumulator to SBUF; the rest of the normalisation pipeline
                # runs off the PE critical path (its deadline is the deferred
                # wo of this strip, a full strip away).
                pv_sb = rp.tile([128, STRIP], BF16, tag="pvs")
                nc.vector.tensor_copy(pv_sb, pv_ps)
                rcb_bc = nrm.tile([128, STRIP], F32, tag="rcbb")
                nc.gpsimd.partition_broadcast(rcb_bc, rcb_row)
                m_sb = rp.tile([128, STRIP], BF16, tag="msb")
                nc.vector.tensor_mul(m_sb, pv_sb, rcb_bc)
                nc.scalar.copy(otile[:, h, 0, :], m_sb)
                nc.vector.tensor_sub(otile[:, h, 1, :], m_sb, otile[:, h, 0, :])

            while j == NSTRIP - 1 and deferred:
                _emit_wo_tile(*deferred.pop(0))
            # wo for this strip is deferred into the next strip's attention
            # window (PE-light there); strip 3 emits inline above.
            if j < NSTRIP - 1:
                deferred.extend((j, otile, nt) for nt in range(NDT))
            else:
                for nt in range(NDT):
                    _emit_wo_tile(j, otile, nt, borrow=True)

    return nc


_PERM = np.concatenate([np.arange(0, HD, 2), np.arange(1, HD, 2)])


def _split_fp8(a):
    """Split f32 array into (hi, lo) fp8e4 planes with hi + lo ~ a."""
    fp8 = ml_dtypes.float8_e4m3
    hi = a.astype(fp8)
    lo = (a - hi.astype(np.float32)).astype(fp8)
    return hi, lo


def _host_prep(x, wq, wk, wv, wo, freqs_cos, freqs_sin, mask):
    bf16 = ml_dtypes.bfloat16
    x = np.asarray(x, np.float32)
    wq = np.asarray(wq, np.float32)
    wk = np.asarray(wk, np.float32)
    wv = np.asarray(wv, np.float32)
    wo = np.asarray(wo, np.float32)
    cos = np.asarray(freqs_cos, np.float32)   # [S, HD/2]
    sin = np.asarray(freqs_sin, np.float32)
    mask = np.asarray(mask, np.float32)

    cosH = cos.T                               # [64, S]
    sinH = sin.T
    cs = np.vstack([cosH, cosH]).astype(bf16)  # [128, S]
    sn = np.vstack([sinH, sinH]).astype(bf16)

    # multiplicative binary causal mask for the 4 diagonal-tile flavours:
    # mk[k, d, q] = 1 where allowed, 0 where masked
    mk = np.empty((SKT, 4, STRIP), np.float32)
    for d_ in range(4):
        sub = mask[0:STRIP, d_ * SKT:(d_ + 1) * SKT]   # [q, k]
        mk[:, d_, :] = np.where(np.isfinite(sub), 1.0, 0.0).T
    mk = mk.astype(bf16)

    perm_g = np.concatenate([h * HD + _PERM for h in range(HPG)])

    # lhsT of the rope pair-mix matmul: (J q) rows 0:64 = -q[64:128],
    # rows 64:128 = +q[0:64]; jt = J.T
    jt = np.zeros((HD, HD), np.float32)
    jt[np.arange(64), np.arange(64) + 64] = 1.0
    jt[np.arange(64) + 64, np.arange(64)] = -1.0
    jt = jt.astype(bf16)

    def _pack_w(wT):
        # wT: [D, EG] f32 -> [128, NDT, 2(lo,hi), EG] fp8, scaled by WSCALE
        hi, lo = _split_fp8(wT * WSCALE)
        st = np.stack([lo, hi], axis=1)            # [D, 2, EG]
        return np.ascontiguousarray(
            st.reshape(NDT, 128, 2, EG).transpose(1, 0, 2, 3))

    def _pack_x(xT):
        # xT: [D, S] f32 -> [128, NDT, 2(hi,lo), S] fp8
        hi, lo = _split_fp8(xT)
        st = np.stack([hi, lo], axis=1)            # [D, 2, S]
        return np.ascontiguousarray(
            st.reshape(NDT, 128, 2, S).transpose(1, 0, 2, 3))

    in_maps = []
    for c in range(NCORES):
        b, g = c // HPG, c % HPG
        rows = slice(g * EG, (g + 1) * EG)
        wq_g = wq[rows][perm_g]                # [EG, D], head dims permuted
        wk_g = wk[rows][perm_g]
        wv_g = wv[rows]
        wo_g = wo[:, rows]                     # [D, EG]
        # woT: [EG, D] -> [128, HPG(h-chunk), 2(lo,hi), D]
        hi, lo = _split_fp8(wo_g.T * WSCALE)
        wo_st = np.stack([lo, hi], axis=1)     # [EG, 2, D]
        wo_pk = np.ascontiguousarray(
            wo_st.reshape(HPG, 128, 2, D).transpose(1, 0, 2, 3))
        in_maps.append({
            "xT": _pack_x(x[b].T),
            "wqT": _pack_w(wq_g.T),
            "wkT": _pack_w(wk_g.T),
            "wvT": _pack_w(wv_g.T),
            "woT": wo_pk,
            "cs": cs, "sn": sn, "mk": mk, "jt": jt,
        })
    return in_maps


def kernel(x, wq, wk, wv, wo, freqs_cos, freqs_sin, mask, start_pos):
    global LAST_EXEC_NS, LAST_RESULTS
    in_maps = _host_prep(x, wq, wk, wv, wo, freqs_cos, freqs_sin, mask)
    nc = _build_program()
    nc.finalize()
    res = run_bass_kernel_spmd(nc, in_maps, core_ids=list(range(NCORES)),
                               trace=False)
    LAST_EXEC_NS = res.exec_time_ns
    LAST_RESULTS = res
    out = np.empty((B, S, D), np.float32)
    for b in range(B):
        acc = np.zeros((D, S), np.float32)
        for g in range(HPG):
            acc += res.results[b * HPG + g]["outT"].astype(np.float32)
        out[b] = acc.T
    return out


# revision 24
# speedup vs baseline: 1.0927x; 1.0089x over previous
"""Self-contained Trainium2 Bass kernel for multi-head causal attention with RoPE.

Problem: B=2, S=2048, D=2048, H=16 heads (HD=128), fp32 reference:
    q = rope(x @ wq.T), k = rope(x @ wk.T), v = x @ wv.T
    out = softmax(q k^T / sqrt(HD) + causal_mask) @ v @ wo.T

Sharding (8 cores): core c = (b, g) with b = c // 4 (batch), g = c % 4
(head-group of 4 heads).  Each core computes its head-group's attention for
its batch and a partial output projection (columns 512g:512g+512 of the
attention output times the matching wo rows).  The host sums the 4 partial
[D, S] tensors per batch and transposes back to [S, D].

Matmul precision: the four big projections (q, k, v, wo) run as fp8e4
DoubleRow matmuls with a 3-term hi/lo split (w*x ~ wh*xh + wl*xh + wh*xl).
DoubleRow contracts two 128-partition planes per pass at half the cycle
cost, so the 3-term split costs 0.75x the bf16 rows while keeping ~11 bits
of mantissa.  x and the weights are split host-side (hi/lo planes are
interleaved in the free dim: x planes ordered (hi, lo), weight planes
(lo, hi) so one cross-term instruction pairs (wl*xh, wh*xl)).  The
attention matmuls (scores, PV) stay bf16 (contraction 128 gains nothing
from DoubleRow).

Softmax denominators come from the PE: each masked-exp tile is used as a
stationary operand against a moving ones-vector, accumulating per-q-chunk
column sums [128q, 1] in a dedicated PSUM bank (cost ~1 row per tile).
The [128, 4] reciprocal is transposed to a [1, 512] row by four tiny
SWDGE (Pool-issued) DMAs and partition-broadcast on GpSimd, then a single
DVE multiply normalizes the PV accumulator.

On-chip layout is "transposed" as in the bf16 version: Q^T/K^T as
[head_dim, seq], scoresT[k, q] feeds PV directly.  RoPE pair mixing is a
PE matmul against a signed permutation J (head dims of wq/wk permuted
host-side).  Causal masking: strictly-above-diagonal tiles skipped,
diagonal tiles at partial width with a multiplicative binary bf16 mask
after exp.
"""

import math

import numpy as np
import ml_dtypes

import concourse.bass as bass
import concourse.bacc as bacc
import concourse.mybir as mybir
from concourse.tile import TileContext
from concourse.bass_utils import run_bass_kernel_spmd
from contextlib import ExitStack

B, S, D, H = 2, 2048, 2048, 16
HD = 128          # head dim
HPG = 4           # heads per core (group)
EG = HPG * HD     # 512 head dims per core
NCORES = 8
NSTRIP = 4        # q strips per sequence
STRIP = S // NSTRIP   # 512
SKT = 128         # k tile (partition dim of scoresT)
NDT = D // 128    # 16 contraction tiles for projections
SCALE = 1.0 / math.sqrt(HD)

BF16 = mybir.dt.bfloat16
F32 = mybir.dt.float32
FP8 = mybir.dt.float8e4
DR = mybir.MatmulPerfMode.DoubleRow

# Host-side weight scaling: the reference weights have sigma ~ 1/sqrt(D) =
# 0.022, whose fp8 lo-residual (~0.0014) would underflow e4m3's subnormal
# floor (2^-9).  Scaling every weight matrix by 2^5 puts both planes in the
# normal range; q*k picks up 2^10 (cancelled in the exp scale) and v*wo
# picks up 2^10 (cancelled in the output-projection copies).
WSCALE = 32.0

LAST_EXEC_NS = None
LAST_RESULTS = None


def _build_program():
    nc = bacc.Bacc("TRN2", target_bir_lowering=False, debug=False,
                   num_devices=NCORES)
    # x / weights are pre-split into fp8 hi+lo planes host-side.
    # x planes: dim2 = (hi, lo); weight planes: dim2 = (lo, hi).
    xT_d = nc.dram_tensor("xT", [128, NDT, 2, S], FP8, kind="ExternalInput").ap()
    wqT_d = nc.dram_tensor("wqT", [128, NDT, 2, EG], FP8, kind="ExternalInput").ap()
    wkT_d = nc.dram_tensor("wkT", [128, NDT, 2, EG], FP8, kind="ExternalInput").ap()
    wvT_d = nc.dram_tensor("wvT", [128, NDT, 2, EG], FP8, kind="ExternalInput").ap()
    woT_d = nc.dram_tensor("woT", [128, HPG, 2, D], FP8, kind="ExternalInput").ap()
    cs_d = nc.dram_tensor("cs", [HD, S], BF16, kind="ExternalInput").ap()
    sn_d = nc.dram_tensor("sn", [HD, S], BF16, kind="ExternalInput").ap()
    mk_d = nc.dram_tensor("mk", [SKT, 4, STRIP], BF16, kind="ExternalInput").ap()
    jt_d = nc.dram_tensor("jt", [HD, HD], BF16, kind="ExternalInput").ap()
    outT_d = nc.dram_tensor("outT", [D, S], BF16, kind="ExternalOutput").ap()

    EXP = mybir.ActivationFunctionType.Exp

    with TileContext(nc) as tc, ExitStack() as ctx:
        wpool = ctx.enter_context(tc.tile_pool(name="wpool", bufs=1))
        kv = ctx.enter_context(tc.tile_pool(name="kv", bufs=1))
        xs = ctx.enter_context(tc.tile_pool(name="xs", bufs=2))
        qs = ctx.enter_context(tc.tile_pool(name="qs", bufs=2))
        rp = ctx.enter_context(tc.tile_pool(name="rp", bufs=2))
        ep = ctx.enter_context(tc.tile_pool(name="ep", bufs=4))
        ot = ctx.enter_context(tc.tile_pool(name="ot", bufs=3))
        po = ctx.enter_context(tc.tile_pool(name="po", bufs=6))
        nrm = ctx.enter_context(tc.tile_pool(name="nrm", bufs=2))
        scps = ctx.enter_context(tc.tile_pool(name="scps", bufs=2, space="PSUM"))
        prps = ctx.enter_context(tc.tile_pool(name="prps", bufs=2, space="PSUM"))
        pvps = ctx.enter_context(tc.tile_pool(name="pvps", bufs=2, space="PSUM"))
        wops = ctx.enter_context(tc.tile_pool(name="wops", bufs=1, space="PSUM"))
        dnps = ctx.enter_context(tc.tile_pool(name="dnps", bufs=1, space="PSUM"))

        # persistent SBUF tensors
        wq_sb = wpool.tile([128, NDT, 2, EG], FP8)
        wk_sb = wpool.tile([128, NDT, 2, EG], FP8)
        wv_sb = wpool.tile([128, NDT, 2, EG], FP8)
        wo_sb = wpool.tile([128, HPG, 2, D], FP8)
        cs_sb = wpool.tile([128, S], BF16)
        sn_sb = wpool.tile([128, S], BF16)
        mk_sb = wpool.tile([128, 4, STRIP], BF16)
        jt_sb = wpool.tile([HD, HD], BF16)
        ones_sb = wpool.tile([128, 1], BF16)
        KT_sb = kv.tile([128, HPG, S], BF16)       # [e, h, sk] rope'd K^T
        V_sb = kv.tile([128, S // 128, EG], BF16)  # [sk, sk_tile, e]

        nc.vector.memset(ones_sb, 1.0)

        xt0 = xs.tile([128, NDT, 2, STRIP], FP8, tag="xt")
        x0r = xT_d[:, :, :, 0:STRIP]

        # --- DMA issue order tuned for the critical path: the first q
        # projection needs wq chunk-pairs + the first x chunk-pairs.
        for a, b in ((0, 2), (2, 4), (4, 8), (8, 12), (12, 16)):
            nc.sync.dma_start(out=wq_sb[:, a:b], in_=wqT_d[:, a:b])
            nc.sync.dma_start(out=xt0[:, a:b], in_=x0r[:, a:b])
        for c0 in range(0, NDT, 4):
            nc.sync.dma_start(out=wk_sb[:, c0:c0 + 4], in_=wkT_d[:, c0:c0 + 4])
        nc.sync.dma_start(out=jt_sb, in_=jt_d)
        nc.sync.dma_start(out=wv_sb[:, 0:4], in_=wvT_d[:, 0:4])
        nc.sync.dma_start(out=cs_sb, in_=cs_d)
        nc.sync.dma_start(out=sn_sb, in_=sn_d)
        for c0 in range(4, NDT, 4):
            nc.sync.dma_start(out=wv_sb[:, c0:c0 + 4], in_=wvT_d[:, c0:c0 + 4])
        nc.sync.dma_start(out=mk_sb, in_=mk_d)
        xt1 = xs.tile([128, NDT, 2, STRIP], FP8, tag="xt")
        nc.sync.dma_start(out=xt1, in_=xT_d[:, :, :, STRIP:2 * STRIP])
        nc.sync.dma_start(out=wo_sb, in_=woT_d)

        def _proj3(out_ps, w_sb, xt, e0, ew, s0=0, sw=STRIP):
            """3-term fp8 DoubleRow accumulation: out_ps [128(e), sw] +=
            w[:, :, :, e0:e0+ew].T @ x[:, :, :, s0:s0+sw] over all 16 chunks.
            Weight planes are (lo, hi); x planes are (hi, lo)."""
            ni = NDT // 2 + NDT
            k = 0
            for t in range(NDT // 2):
                nc.tensor.matmul(out_ps,
                                 lhsT=w_sb[:, 2*t:2*t+2, 1, e0:e0+ew],
                                 rhs=xt[:, 2*t:2*t+2, 0, s0:s0+sw],
                                 start=(k == 0), stop=(k == ni - 1),
                                 perf_mode=DR)
                k += 1
            for c in range(NDT):
                nc.tensor.matmul(out_ps,
                                 lhsT=w_sb[:, c, :, e0:e0+ew],
                                 rhs=xt[:, c, :, s0:s0+sw],
                                 start=(k == 0), stop=(k == ni - 1),
                                 perf_mode=DR)
                k += 1

        def _projv3(out_ps, xt, st):
            """V projection: stationary = x chunk [128, 2, 128(s)],
            moving = wv planes [128, 2, 512(e)]."""
            ni = NDT // 2 + NDT
            k = 0
            sl = slice(st * 128, (st + 1) * 128)
            for t in range(NDT // 2):
                nc.tensor.matmul(out_ps,
                                 lhsT=xt[:, 2*t:2*t+2, 0, sl],
                                 rhs=wv_sb[:, 2*t:2*t+2, 1, :],
                                 start=(k == 0), stop=(k == ni - 1),
                                 perf_mode=DR)
                k += 1
            for c in range(NDT):
                nc.tensor.matmul(out_ps,
                                 lhsT=xt[:, c, :, sl],
                                 rhs=wv_sb[:, c, :, :],
                                 start=(k == 0), stop=(k == ni - 1),
                                 perf_mode=DR)
                k += 1

        deferred = []
        COPY = mybir.ActivationFunctionType.Copy
        UNSCALE = 1.0 / (WSCALE * WSCALE)

        def _emit_wo_tile(j, otile, nt, borrow=False):
            """One partial-output-projection tile: 3-term fp8 DR over the
            4 head chunks.  otile: [128(e), HPG, 2(hi,lo), STRIP] fp8.
            The copies also undo the host-side 2^10 weight scaling."""
            s0 = j * STRIP
            nsl = slice(nt * 128, (nt + 1) * 128)
            if borrow:
                # attention is over: rotate across every psum pool so the
                # tail pipelines instead of serialising on one bank
                r = nt % 4
                if r == 0:
                    pr = wops.tile([128, STRIP], F32, tag="pr")
                elif r == 1:
                    pr = scps.tile([128, STRIP], F32, tag="sc")
                elif r == 2:
                    pr = prps.tile([128, STRIP], F32, tag="mm")
                else:
                    pr = pvps.tile([128, STRIP], F32, tag="pv")
            else:
                pr = wops.tile([128, STRIP], F32, tag="pr")
            k = 0
            for t in range(HPG // 2):
                nc.tensor.matmul(pr,
                                 lhsT=wo_sb[:, 2*t:2*t+2, 1, nsl],
                                 rhs=otile[:, 2*t:2*t+2, 0, :],
                                 start=(k == 0), stop=False,
                                 perf_mode=DR)
                k += 1
            for h in range(HPG):
                nc.tensor.matmul(pr,
                                 lhsT=wo_sb[:, h, :, nsl],
                                 rhs=otile[:, h, :, :],
                                 start=False, stop=(h == HPG - 1),
                                 perf_mode=DR)
            pr_sb = po.tile([128, STRIP], BF16, tag="po")
            if nt % 2 == 1:
                nc.scalar.activation(pr_sb, pr, COPY, scale=UNSCALE)
            else:
                nc.vector.tensor_scalar_mul(pr_sb, pr, UNSCALE)
            nc.sync.dma_start(
                out=outT_d[nt * 128:(nt + 1) * 128, s0:s0 + STRIP], in_=pr_sb)

        for j in range(NSTRIP):
            s0 = j * STRIP
            if j == 0:
                xt = xt0
            elif j == 1:
                xt = xt1
            else:
                xt = xs.tile([128, NDT, 2, STRIP], FP8, tag="xt")
                nc.sync.dma_start(out=xt, in_=xT_d[:, :, :, s0:s0 + STRIP])
            qt = qs.tile([128, HPG, STRIP], BF16, tag="qt")

            # --- projections + RoPE for this strip ---
            if j == 0:
                # strip 0: chunk-outer over the 2-chunk DMA pairs so the first
                # matmuls start as soon as (wq pair 0, x pair 0) land; the
                # 4 concurrent head accumulators borrow the idle psum bufs
                qp0 = prps.tile([128, STRIP], F32, tag="mm")
                qp1 = prps.tile([128, STRIP], F32, tag="mm")
                qp2 = scps.tile([128, STRIP], F32, tag="sc")
                qp3 = scps.tile([128, STRIP], F32, tag="sc")
                qps0 = [qp0, qp1, qp2, qp3]
                ni = NDT // 2 + NDT
                kk = [0] * HPG
                for t in range(NDT // 2):
                    for h in range(HPG):
                        e0 = h * HD
                        nc.tensor.matmul(qps0[h],
                                         lhsT=wq_sb[:, 2*t:2*t+2, 1, e0:e0+HD],
                                         rhs=xt[:, 2*t:2*t+2, 0, :],
                                         start=(kk[h] == 0), stop=False,
                                         perf_mode=DR)
                        kk[h] += 1
                        for c in (2*t, 2*t+1):
                            nc.tensor.matmul(qps0[h],
                                             lhsT=wq_sb[:, c, :, e0:e0+HD],
                                             rhs=xt[:, c, :, :],
                                             start=False, stop=(kk[h] == ni - 1),
                                             perf_mode=DR)
                            kk[h] += 1
            # Projections run two "stages" ahead of the J/RoPE mix: the J
            # matmul for head h-1 is emitted after head h's Q/K accumulation,
            # so the PE never head-of-line blocks on the ACT psum->sbuf copy.
            qk_sb = {}

            def _jrope(h):
                for which, dst in (("q", qt[:, h, :]),
                                   ("k", KT_sb[:, h, s0:s0 + STRIP])):
                    src = qk_sb.pop((h, which))
                    if j == 0:
                        jps = pvps.tile([128, STRIP], F32, tag="pv")
                    else:
                        jps = scps.tile([128, STRIP], F32, tag="sc")
                    nc.tensor.matmul(jps, lhsT=jt_sb, rhs=src,
                                     start=True, stop=True)
                    j_sb = rp.tile([128, STRIP], BF16, tag="jsb")
                    nc.scalar.copy(j_sb, jps)
                    t1 = rp.tile([128, STRIP], BF16, tag="ra")
                    nc.vector.tensor_mul(t1, src, cs_sb[:, s0:s0 + STRIP])
                    t2 = rp.tile([128, STRIP], BF16, tag="rb")
                    nc.vector.tensor_mul(t2, j_sb, sn_sb[:, s0:s0 + STRIP])
                    nc.vector.tensor_add(dst, t1, t2)

            for h in range(HPG):
                e0 = h * HD
                if j == 0:
                    q_ps = qps0[h]
                else:
                    q_ps = prps.tile([128, STRIP], F32, tag="mm")
                    _proj3(q_ps, wq_sb, xt, e0, HD)
                q_sb = rp.tile([128, STRIP], BF16, tag="qsb", bufs=4)
                nc.scalar.copy(q_sb, q_ps)
                qk_sb[(h, "q")] = q_sb

                k_ps = prps.tile([128, STRIP], F32, tag="mm")
                _proj3(k_ps, wk_sb, xt, e0, HD)
                k_sb = rp.tile([128, STRIP], BF16, tag="ksb", bufs=4)
                nc.vector.tensor_copy(k_sb, k_ps)
                qk_sb[(h, "k")] = k_sb
                if h > 0:
                    _jrope(h - 1)

            for st in range(4):
                v_ps = prps.tile([128, EG], F32, tag="mm")
                _projv3(v_ps, xt, st)
                nc.vector.tensor_copy(V_sb[:, j * 4 + st, :], v_ps)
                if st == 0:
                    _jrope(HPG - 1)

            # --- attention for this strip ---
            otile = ot.tile([128, HPG, 2, STRIP], FP8, tag="ot")
            nsk = 4 * j + 4
            for h in range(HPG):
                # previous strip's deferred wo tiles, 4 per head boundary
                for _ in range(4):
                    if deferred:
                        _emit_wo_tile(*deferred.pop(0))
                e0 = h * HD
                pv_ps = pvps.tile([128, STRIP], F32, tag="pv")
                den_ps = dnps.tile([128, 4], F32, tag="dn")
                rcb_c = nrm.tile([128, 4], F32, tag="rcbc")
                rcb_row = nrm.tile([1, STRIP], F32, tag="rcbr")
                for skt in range(nsk):
                    d = skt - 4 * j   # >= 0 on the diagonal block
                    w = STRIP - 128 * d if d > 0 else STRIP
                    dd = max(d, 0)
                    first = (skt == 0)
                    sc_ps = scps.tile([128, STRIP], F32, tag="sc")
                    nc.tensor.matmul(sc_ps[:, 0:w],
                                     lhsT=KT_sb[:, h, skt * 128:(skt + 1) * 128],
                                     rhs=qt[:, h, STRIP - w:STRIP],
                                     start=True, stop=True)
                    ex = ep.tile([128, STRIP], BF16, tag="ex")
                    nc.scalar.activation(ex[:, 0:w], sc_ps[:, 0:w], EXP,
                                         scale=SCALE / (WSCALE * WSCALE))
                    if d >= 0:
                        exm = ep.tile([128, STRIP], BF16, tag="exm")
                        nc.vector.tensor_mul(exm[:, 0:w], ex[:, 0:w],
                                             mk_sb[:, d, STRIP - w:STRIP])
                    else:
                        exm = ex
                    nc.tensor.matmul(pv_ps[:, STRIP - w:STRIP],
                                     lhsT=V_sb[:, skt, e0:e0 + HD],
                                     rhs=exm[:, 0:w], start=first,
                                     stop=(skt == nsk - 1))
                    # denominator columns: exm chunk as stationary x ones.
                    # NOTE: start=True zeroes the whole PSUM bank, so only the
                    # very first den matmul of the head-strip starts the bank;
                    # every column then accumulates into the zeroed bank.
                    for qc in range(dd, 4):
                        last = (skt == nsk - 1 - (3 - qc))
                        nc.tensor.matmul(
                            den_ps[:, qc:qc + 1],
                            lhsT=exm[:, (qc - dd) * 128:(qc - dd + 1) * 128],
                            rhs=ones_sb,
                            start=(first and qc == 0),
                            stop=last,
                            skip_group_check=True)
                        if last:
                            nc.vector.reciprocal(rcb_c[:, qc:qc + 1],
                                                 den_ps[:, qc:qc + 1])
                            nc.gpsimd.dma_start(
                                out=rcb_row[:, qc * 128:(qc + 1) * 128],
                                in_=rcb_c[:, qc:qc + 1])
                # free the pv psum bank quickly: copy the unnormalised
                # accumulator to SBUF; the rest of the normalisation pipeline
                # runs off the PE critical path (its deadline is the deferred
                # wo of this strip, a full strip away).
                pv_sb = rp.tile([128, STRIP], BF16, tag="pvs")
                nc.vector.tensor_copy(pv_sb, pv_ps)
                rcb_bc = nrm.tile([128, STRIP], F32, tag="rcbb")
                nc.gpsimd.partition_broadcast(rcb_bc, rcb_row)
                m_sb = rp.tile([128, STRIP], BF16, tag="msb")
                nc.vector.tensor_mul(m_sb, pv_sb, rcb_bc)
                nc.scalar.copy(otile[:, h, 0, :], m_sb)
                nc.vector.tensor_sub(otile[:, h, 1, :], m_sb, otile[:, h, 0, :])

            while j == NSTRIP - 1 and deferred:
                _emit_wo_tile(*deferred.pop(0))
            # wo for this strip is deferred into the next strip's attention
            # window (PE-light there); strip 3 emits inline above.
            if j < NSTRIP - 1:
                deferred.extend((j, otile, nt) for nt in range(NDT))
            else:
                for nt in range(NDT):
                    _emit_wo_tile(j, otile, nt, borrow=True)

    return nc


_PERM = np.concatenate([np.arange(0, HD, 2), np.arange(1, HD, 2)])


def _split_fp8(a):
    """Split f32 array into (hi, lo) fp8e4 planes with hi + lo ~ a."""
    fp8 = ml_dtypes.float8_e4m3
    hi = a.astype(fp8)
    lo = (a - hi.astype(np.float32)).astype(fp8)
    return hi, lo


def _host_prep(x, wq, wk, wv, wo, freqs_cos, freqs_sin, mask):
    bf16 = ml_dtypes.bfloat16
    x = np.asarray(x, np.float32)
    wq = np.asarray(wq, np.float32)
    wk = np.asarray(wk, np.float32)
    wv = np.asarray(wv, np.float32)
    wo = np.asarray(wo, np.float32)
    cos = np.asarray(freqs_cos, np.float32)   # [S, HD/2]
    sin = np.asarray(freqs_sin, np.float32)
    mask = np.asarray(mask, np.float32)

    cosH = cos.T                               # [64, S]
    sinH = sin.T
    cs = np.vstack([cosH, cosH]).astype(bf16)  # [128, S]
    sn = np.vstack([sinH, sinH]).astype(bf16)

    # multiplicative binary causal mask for the 4 diagonal-tile flavours:
    # mk[k, d, q] = 1 where allowed, 0 where masked
    mk = np.empty((SKT, 4, STRIP), np.float32)
    for d_ in range(4):
        sub = mask[0:STRIP, d_ * SKT:(d_ + 1) * SKT]   # [q, k]
        mk[:, d_, :] = np.where(np.isfinite(sub), 1.0, 0.0).T
    mk = mk.astype(bf16)

    perm_g = np.concatenate([h * HD + _PERM for h in range(HPG)])

    # lhsT of the rope pair-mix matmul: (J q) rows 0:64 = -q[64:128],
    # rows 64:128 = +q[0:64]; jt = J.T
    jt = np.zeros((HD, HD), np.float32)
    jt[np.arange(64), np.arange(64) + 64] = 1.0
    jt[np.arange(64) + 64, np.arange(64)] = -1.0
    jt = jt.astype(bf16)

    def _pack_w(wT):
        # wT: [D, EG] f32 -> [128, NDT, 2(lo,hi), EG] fp8, scaled by WSCALE
        hi, lo = _split_fp8(wT * WSCALE)
        st = np.stack([lo, hi], axis=1)            # [D, 2, EG]
        return np.ascontiguousarray(
            st.reshape(NDT, 128, 2, EG).transpose(1, 0, 2, 3))

    def _pack_x(xT):
        # xT: [D, S] f32 -> [128, NDT, 2(hi,lo), S] fp8
        hi, lo = _split_fp8(xT)
        st = np.stack([hi, lo], axis=1)            # [D, 2, S]
        return np.ascontiguousarray(
            st.reshape(NDT, 128, 2, S).transpose(1, 0, 2, 3))

    in_maps = []
    for c in range(NCORES):
        b, g = c // HPG, c % HPG
        rows = slice(g * EG, (g + 1) * EG)
        wq_g = wq[rows][perm_g]                # [EG, D], head dims permuted
        wk_g = wk[rows][perm_g]
        wv_g = wv[rows]
        wo_g = wo[:, rows]                     # [D, EG]
        # woT: [EG, D] -> [128, HPG(h-chunk), 2(lo,hi), D]
        hi, lo = _split_fp8(wo_g.T * WSCALE)
        wo_st = np.stack([lo, hi], axis=1)     # [EG, 2, D]
        wo_pk = np.ascontiguousarray(
            wo_st.reshape(HPG, 128, 2, D).transpose(1, 0, 2, 3))
        in_maps.append({
            "xT": _pack_x(x[b].T),
            "wqT": _pack_w(wq_g.T),
            "wkT": _pack_w(wk_g.T),
            "wvT": _pack_w(wv_g.T),
            "woT": wo_pk,
            "cs": cs, "sn": sn, "mk": mk, "jt": jt,
        })
    return in_maps


def kernel(x, wq, wk, wv, wo, freqs_cos, freqs_sin, mask, start_pos):
    global LAST_EXEC_NS, LAST_RESULTS
    in_maps = _host_prep(x, wq, wk, wv, wo, freqs_cos, freqs_sin, mask)
    nc = _build_program()
    nc.finalize()
    res = run_bass_kernel_spmd(nc, in_maps, core_ids=list(range(NCORES)),
                               trace=False)
    LAST_EXEC_NS = res.exec_time_ns
    LAST_RESULTS = res
    out = np.empty((B, S, D), np.float32)
    for b in range(B):
        acc = np.zeros((D, S), np.float32)
        for g in range(HPG):
            acc += res.results[b * HPG + g]["outT"].astype(np.float32)
        out[b] = acc.T
    return out


# revision 26
# speedup vs baseline: 1.1367x; 1.0403x over previous
"""Self-contained Trainium2 Bass kernel for multi-head causal attention with RoPE.

Problem: B=2, S=2048, D=2048, H=16 heads (HD=128), fp32 reference:
    q = rope(x @ wq.T), k = rope(x @ wk.T), v = x @ wv.T
    out = softmax(q k^T / sqrt(HD) + causal_mask) @ v @ wo.T

Sharding (8 cores): core c = (b, g) with b = c // 4 (batch), g = c % 4
(head-group of 4 heads).  Each core computes its head-group's attention for
its batch and a partial output projection (columns 512g:512g+512 of the
attention output times the matching wo rows).  The host sums the 4 partial
[D, S] tensors per batch and transposes back to [S, D].

Matmul precision: the four big projections (q, k, v, wo) run as fp8e4
DoubleRow matmuls with a 3-term hi/lo split (w*x ~ wh*xh + wl*xh + wh*xl).
DoubleRow contracts two 128-partition planes per pass at half the cycle
cost, so the 3-term split costs 0.75x the bf16 rows while keeping ~11 bits
of mantissa.  x and the weights are split host-side (hi/lo planes are
interleaved in the free dim: x planes ordered (hi, lo), weight planes
(lo, hi) so one cross-term instruction pairs (wl*xh, wh*xl)).  The
attention matmuls (scores, PV) stay bf16 (contraction 128 gains nothing
from DoubleRow).

Softmax denominators come from the PE: each masked-exp tile is used as a
stationary operand against a moving ones-vector, accumulating per-q-chunk
column sums [128q, 1] in a dedicated PSUM bank (cost ~1 row per tile).
The [128, 4] reciprocal is transposed to a [1, 512] row by four tiny
SWDGE (Pool-issued) DMAs and partition-broadcast on GpSimd, then a single
DVE multiply normalizes the PV accumulator.

On-chip layout is "transposed" as in the bf16 version: Q^T/K^T as
[head_dim, seq], scoresT[k, q] feeds PV directly.  RoPE pair mixing is a
PE matmul against a signed permutation J (head dims of wq/wk permuted
host-side).  Causal masking: strictly-above-diagonal tiles skipped,
diagonal tiles at partial width with a multiplicative binary bf16 mask
after exp.
"""

import math

import numpy as np
import ml_dtypes

import concourse.bass as bass
import concourse.bacc as bacc
import concourse.mybir as mybir
from concourse.tile import TileContext
from concourse.bass_utils import run_bass_kernel_spmd
from contextlib import ExitStack

B, S, D, H = 2, 2048, 2048, 16
HD = 128          # head dim
HPG = 4           # heads per core (group)
EG = HPG * HD     # 512 head dims per core
NCORES = 8
NSTRIP = 4        # q strips per sequence
STRIP = S // NSTRIP   # 512
SKT = 128         # k tile (partition dim of scoresT)
NDT = D // 128    # 16 contraction tiles for projections
SCALE = 1.0 / math.sqrt(HD)

BF16 = mybir.dt.bfloat16
F32 = mybir.dt.float32
FP8 = mybir.dt.float8e4
DR = mybir.MatmulPerfMode.DoubleRow

# Host-side weight scaling: the reference weights have sigma ~ 1/sqrt(D) =
# 0.022, whose fp8 lo-residual (~0.0014) would underflow e4m3's subnormal
# floor (2^-9).  Scaling every weight matrix by 2^5 puts both planes in the
# normal range; q*k picks up 2^10 (cancelled in the exp scale) and v*wo
# picks up 2^10 (cancelled in the output-projection copies).
WSCALE = 32.0

LAST_EXEC_NS = None
LAST_RESULTS = None

import os
V_PR_DVE = int(os.environ.get("V_PR_DVE", "1"))      # pr_sb copies: 0=alt, 1=all DVE, 2=all ACT
V_OT_DVE = int(os.environ.get("V_OT_DVE", "2"))      # otile-hi copy on DVE for j >= this (4=never)
V_QSB_DVE = int(os.environ.get("V_QSB_DVE", "1"))    # q_sb copy engine: 0=ACT, 1=DVE
V_KSB_ACT = int(os.environ.get("V_KSB_ACT", "0"))    # k_sb copy engine: 0=DVE, 1=ACT
V_EP_BUFS = int(os.environ.get("V_EP_BUFS", "6"))
V_PO_BUFS = int(os.environ.get("V_PO_BUFS", "6"))
V_WO_PER_HEAD = int(os.environ.get("V_WO_PER_HEAD", "3"))


def _build_program():
    nc = bacc.Bacc("TRN2", target_bir_lowering=False, debug=False,
                   num_devices=NCORES)
    # x / weights are pre-split into fp8 hi+lo planes host-side.
    # x planes: dim2 = (hi, lo); weight planes: dim2 = (lo, hi).
    xT_d = nc.dram_tensor("xT", [128, NDT, 2, S], FP8, kind="ExternalInput").ap()
    wqT_d = nc.dram_tensor("wqT", [128, NDT, 2, EG], FP8, kind="ExternalInput").ap()
    wkT_d = nc.dram_tensor("wkT", [128, NDT, 2, EG], FP8, kind="ExternalInput").ap()
    wvT_d = nc.dram_tensor("wvT", [128, NDT, 2, EG], FP8, kind="ExternalInput").ap()
    woT_d = nc.dram_tensor("woT", [128, HPG, 2, D], FP8, kind="ExternalInput").ap()
    cs_d = nc.dram_tensor("cs", [HD, S], BF16, kind="ExternalInput").ap()
    sn_d = nc.dram_tensor("sn", [HD, S], BF16, kind="ExternalInput").ap()
    mk_d = nc.dram_tensor("mk", [SKT, 4, STRIP], BF16, kind="ExternalInput").ap()
    jt_d = nc.dram_tensor("jt", [HD, HD], BF16, kind="ExternalInput").ap()
    outT_d = nc.dram_tensor("outT", [D, S], BF16, kind="ExternalOutput").ap()

    EXP = mybir.ActivationFunctionType.Exp

    with TileContext(nc) as tc, ExitStack() as ctx:
        wpool = ctx.enter_context(tc.tile_pool(name="wpool", bufs=1))
        kv = ctx.enter_context(tc.tile_pool(name="kv", bufs=1))
        xs = ctx.enter_context(tc.tile_pool(name="xs", bufs=2))
        qs = ctx.enter_context(tc.tile_pool(name="qs", bufs=2))
        rp = ctx.enter_context(tc.tile_pool(name="rp", bufs=2))
        ep = ctx.enter_context(tc.tile_pool(name="ep", bufs=V_EP_BUFS))
        ot = ctx.enter_context(tc.tile_pool(name="ot", bufs=3))
        po = ctx.enter_context(tc.tile_pool(name="po", bufs=V_PO_BUFS))
        nrm = ctx.enter_context(tc.tile_pool(name="nrm", bufs=2))
        scps = ctx.enter_context(tc.tile_pool(name="scps", bufs=2, space="PSUM"))
        prps = ctx.enter_context(tc.tile_pool(name="prps", bufs=2, space="PSUM"))
        pvps = ctx.enter_context(tc.tile_pool(name="pvps", bufs=2, space="PSUM"))
        wops = ctx.enter_context(tc.tile_pool(name="wops", bufs=1, space="PSUM"))
        dnps = ctx.enter_context(tc.tile_pool(name="dnps", bufs=1, space="PSUM"))

        # persistent SBUF tensors
        wq_sb = wpool.tile([128, NDT, 2, EG], FP8)
        wk_sb = wpool.tile([128, NDT, 2, EG], FP8)
        wv_sb = wpool.tile([128, NDT, 2, EG], FP8)
        wo_sb = wpool.tile([128, HPG, 2, D], FP8)
        cs_sb = wpool.tile([128, S], BF16)
        sn_sb = wpool.tile([128, S], BF16)
        mk_sb = wpool.tile([128, 4, STRIP], BF16)
        jt_sb = wpool.tile([HD, HD], BF16)
        ones_sb = wpool.tile([128, 1], BF16)
        KT_sb = kv.tile([128, HPG, S], BF16)       # [e, h, sk] rope'd K^T
        V_sb = kv.tile([128, S // 128, EG], BF16)  # [sk, sk_tile, e]

        nc.vector.memset(ones_sb, 1.0)

        xt0 = xs.tile([128, NDT, 2, STRIP], FP8, tag="xt")
        x0r = xT_d[:, :, :, 0:STRIP]

        # --- DMA issue order tuned for the critical path: the first q
        # projection needs wq chunk-pairs + the first x chunk-pairs.
        for a, b in ((0, 2), (2, 4), (4, 8), (8, 12), (12, 16)):
            nc.sync.dma_start(out=wq_sb[:, a:b], in_=wqT_d[:, a:b])
            nc.sync.dma_start(out=xt0[:, a:b], in_=x0r[:, a:b])
        for c0 in range(0, NDT, 4):
            nc.sync.dma_start(out=wk_sb[:, c0:c0 + 4], in_=wkT_d[:, c0:c0 + 4])
        nc.sync.dma_start(out=jt_sb, in_=jt_d)
        nc.sync.dma_start(out=wv_sb[:, 0:4], in_=wvT_d[:, 0:4])
        nc.sync.dma_start(out=cs_sb, in_=cs_d)
        nc.sync.dma_start(out=sn_sb, in_=sn_d)
        for c0 in range(4, NDT, 4):
            nc.sync.dma_start(out=wv_sb[:, c0:c0 + 4], in_=wvT_d[:, c0:c0 + 4])
        nc.sync.dma_start(out=mk_sb, in_=mk_d)
        xt1 = xs.tile([128, NDT, 2, STRIP], FP8, tag="xt")
        nc.sync.dma_start(out=xt1, in_=xT_d[:, :, :, STRIP:2 * STRIP])
        nc.sync.dma_start(out=wo_sb, in_=woT_d)

        def _proj3(out_ps, w_sb, xt, e0, ew, s0=0, sw=STRIP):
            """3-term fp8 DoubleRow accumulation: out_ps [128(e), sw] +=
            w[:, :, :, e0:e0+ew].T @ x[:, :, :, s0:s0+sw] over all 16 chunks.
            Weight planes are (lo, hi); x planes are (hi, lo)."""
            ni = NDT // 2 + NDT
            k = 0
            for t in range(NDT // 2):
                nc.tensor.matmul(out_ps,
                                 lhsT=w_sb[:, 2*t:2*t+2, 1, e0:e0+ew],
                                 rhs=xt[:, 2*t:2*t+2, 0, s0:s0+sw],
                                 start=(k == 0), stop=(k == ni - 1),
                                 perf_mode=DR)
                k += 1
            for c in range(NDT):
                nc.tensor.matmul(out_ps,
                                 lhsT=w_sb[:, c, :, e0:e0+ew],
                                 rhs=xt[:, c, :, s0:s0+sw],
                                 start=(k == 0), stop=(k == ni - 1),
                                 perf_mode=DR)
                k += 1

        def _projv3(out_ps, xt, st):
            """V projection: stationary = x chunk [128, 2, 128(s)],
            moving = wv planes [128, 2, 512(e)]."""
            ni = NDT // 2 + NDT
            k = 0
            sl = slice(st * 128, (st + 1) * 128)
            for t in range(NDT // 2):
                nc.tensor.matmul(out_ps,
                                 lhsT=xt[:, 2*t:2*t+2, 0, sl],
                                 rhs=wv_sb[:, 2*t:2*t+2, 1, :],
                                 start=(k == 0), stop=(k == ni - 1),
                                 perf_mode=DR)
                k += 1
            for c in range(NDT):
                nc.tensor.matmul(out_ps,
                                 lhsT=xt[:, c, :, sl],
                                 rhs=wv_sb[:, c, :, :],
                                 start=(k == 0), stop=(k == ni - 1),
                                 perf_mode=DR)
                k += 1

        deferred = []
        COPY = mybir.ActivationFunctionType.Copy
        UNSCALE = 1.0 / (WSCALE * WSCALE)

        def _emit_wo_tile(j, otile, nt, borrow=False):
            """One partial-output-projection tile: 3-term fp8 DR over the
            4 head chunks.  otile: [128(e), HPG, 2(hi,lo), STRIP] fp8.
            The copies also undo the host-side 2^10 weight scaling."""
            s0 = j * STRIP
            nsl = slice(nt * 128, (nt + 1) * 128)
            if borrow:
                # attention is over: rotate across every psum pool so the
                # tail pipelines instead of serialising on one bank
                r = nt % 4
                if r == 0:
                    pr = wops.tile([128, STRIP], F32, tag="pr")
                elif r == 1:
                    pr = scps.tile([128, STRIP], F32, tag="sc")
                elif r == 2:
                    pr = prps.tile([128, STRIP], F32, tag="mm")
                else:
                    pr = pvps.tile([128, STRIP], F32, tag="pv")
            else:
                pr = wops.tile([128, STRIP], F32, tag="pr")
            k = 0
            for t in range(HPG // 2):
                nc.tensor.matmul(pr,
                                 lhsT=wo_sb[:, 2*t:2*t+2, 1, nsl],
                                 rhs=otile[:, 2*t:2*t+2, 0, :],
                                 start=(k == 0), stop=False,
                                 perf_mode=DR)
                k += 1
            for h in range(HPG):
                nc.tensor.matmul(pr,
                                 lhsT=wo_sb[:, h, :, nsl],
                                 rhs=otile[:, h, :, :],
                                 start=False, stop=(h == HPG - 1),
                                 perf_mode=DR)
            pr_sb = po.tile([128, STRIP], BF16, tag="po")
            use_act = (nt % 2 == 1) if V_PR_DVE == 0 else (V_PR_DVE == 2)
            if use_act:
                nc.scalar.activation(pr_sb, pr, COPY, scale=UNSCALE)
            else:
                nc.vector.tensor_scalar_mul(pr_sb, pr, UNSCALE)
            nc.sync.dma_start(
                out=outT_d[nt * 128:(nt + 1) * 128, s0:s0 + STRIP], in_=pr_sb)

        for j in range(NSTRIP):
            s0 = j * STRIP
            if j == 0:
                xt = xt0
            elif j == 1:
                xt = xt1
            else:
                xt = xs.tile([128, NDT, 2, STRIP], FP8, tag="xt")
                nc.sync.dma_start(out=xt, in_=xT_d[:, :, :, s0:s0 + STRIP])
            qt = qs.tile([128, HPG, STRIP], BF16, tag="qt")

            # --- projections + RoPE for this strip ---
            if j == 0:
                # strip 0: chunk-outer over the 2-chunk DMA pairs so the first
                # matmuls start as soon as (wq pair 0, x pair 0) land; the
                # 4 concurrent head accumulators borrow the idle psum bufs
                qp0 = prps.tile([128, STRIP], F32, tag="mm")
                qp1 = prps.tile([128, STRIP], F32, tag="mm")
                qp2 = scps.tile([128, STRIP], F32, tag="sc")
                qp3 = scps.tile([128, STRIP], F32, tag="sc")
                qps0 = [qp0, qp1, qp2, qp3]
                ni = NDT // 2 + NDT
                kk = [0] * HPG
                for t in range(NDT // 2):
                    for h in range(HPG):
                        e0 = h * HD
                        nc.tensor.matmul(qps0[h],
                                         lhsT=wq_sb[:, 2*t:2*t+2, 1, e0:e0+HD],
                                         rhs=xt[:, 2*t:2*t+2, 0, :],
                                         start=(kk[h] == 0), stop=False,
                                         perf_mode=DR)
                        kk[h] += 1
                        for c in (2*t, 2*t+1):
                            nc.tensor.matmul(qps0[h],
                                             lhsT=wq_sb[:, c, :, e0:e0+HD],
                                             rhs=xt[:, c, :, :],
                                             start=False, stop=(kk[h] == ni - 1),
                                             perf_mode=DR)
                            kk[h] += 1
            # Projections run two "stages" ahead of the J/RoPE mix: the J
            # matmul for head h-1 is emitted after head h's Q/K accumulation,
            # so the PE never head-of-line blocks on the ACT psum->sbuf copy.
            qk_sb = {}

            def _jrope(h):
                for which, dst in (("q", qt[:, h, :]),
                                   ("k", KT_sb[:, h, s0:s0 + STRIP])):
                    src = qk_sb.pop((h, which))
                    if j == 0:
                        jps = pvps.tile([128, STRIP], F32, tag="pv")
                    else:
                        jps = scps.tile([128, STRIP], F32, tag="sc")
                    nc.tensor.matmul(jps, lhsT=jt_sb, rhs=src,
                                     start=True, stop=True)
                    j_sb = rp.tile([128, STRIP], BF16, tag="jsb")
                    nc.scalar.copy(j_sb, jps)
                    t1 = rp.tile([128, STRIP], BF16, tag="ra")
                    nc.vector.tensor_mul(t1, src, cs_sb[:, s0:s0 + STRIP])
                    t2 = rp.tile([128, STRIP], BF16, tag="rb")
                    nc.vector.tensor_mul(t2, j_sb, sn_sb[:, s0:s0 + STRIP])
                    nc.vector.tensor_add(dst, t1, t2)

            for h in range(HPG):
                e0 = h * HD
                if j == 0:
                    q_ps = qps0[h]
                else:
                    q_ps = prps.tile([128, STRIP], F32, tag="mm")
                    _proj3(q_ps, wq_sb, xt, e0, HD)
                q_sb = rp.tile([128, STRIP], BF16, tag="qsb", bufs=4)
                if V_QSB_DVE:
                    nc.vector.tensor_copy(q_sb, q_ps)
                else:
                    nc.scalar.copy(q_sb, q_ps)
                qk_sb[(h, "q")] = q_sb

                k_ps = prps.tile([128, STRIP], F32, tag="mm")
                _proj3(k_ps, wk_sb, xt, e0, HD)
                k_sb = rp.tile([128, STRIP], BF16, tag="ksb", bufs=4)
                if V_KSB_ACT:
                    nc.scalar.copy(k_sb, k_ps)
                else:
                    nc.vector.tensor_copy(k_sb, k_ps)
                qk_sb[(h, "k")] = k_sb
                if h > 0:
                    _jrope(h - 1)

            for st in range(4):
                v_ps = prps.tile([128, EG], F32, tag="mm")
                _projv3(v_ps, xt, st)
                nc.vector.tensor_copy(V_sb[:, j * 4 + st, :], v_ps)
                if st == 0:
                    _jrope(HPG - 1)

            # --- attention for this strip ---
            otile = ot.tile([128, HPG, 2, STRIP], FP8, tag="ot")
            nsk = 4 * j + 4
            for h in range(HPG):
                # previous strip's deferred wo tiles, 4 per head boundary
                for _ in range(V_WO_PER_HEAD):
                    if deferred:
                        _emit_wo_tile(*deferred.pop(0))
                e0 = h * HD
                pv_ps = pvps.tile([128, STRIP], F32, tag="pv")
                den_ps = dnps.tile([128, 4], F32, tag="dn")
                rcb_c = nrm.tile([128, 4], F32, tag="rcbc")
                rcb_row = nrm.tile([1, STRIP], F32, tag="rcbr")
                for skt in range(nsk):
                    d = skt - 4 * j   # >= 0 on the diagonal block
                    w = STRIP - 128 * d if d > 0 else STRIP
                    dd = max(d, 0)
                    first = (skt == 0)
                    sc_ps = scps.tile([128, STRIP], F32, tag="sc")
                    nc.tensor.matmul(sc_ps[:, 0:w],
                                     lhsT=KT_sb[:, h, skt * 128:(skt + 1) * 128],
                                     rhs=qt[:, h, STRIP - w:STRIP],
                                     start=True, stop=True)
                    ex = ep.tile([128, STRIP], BF16, tag="ex")
                    nc.scalar.activation(ex[:, 0:w], sc_ps[:, 0:w], EXP,
                                         scale=SCALE / (WSCALE * WSCALE))
                    if d >= 0:
                        exm = ep.tile([128, STRIP], BF16, tag="exm")
                        nc.vector.tensor_mul(exm[:, 0:w], ex[:, 0:w],
                                             mk_sb[:, d, STRIP - w:STRIP])
                    else:
                        exm = ex
                    nc.tensor.matmul(pv_ps[:, STRIP - w:STRIP],
                                     lhsT=V_sb[:, skt, e0:e0 + HD],
                                     rhs=exm[:, 0:w], start=first,
                                     stop=(skt == nsk - 1))
                    # denominator columns: exm chunk as stationary x ones.
                    # NOTE: start=True zeroes the whole PSUM bank, so only the
                    # very first den matmul of the head-strip starts the bank;
                    # every column then accumulates into the zeroed bank.
                    for qc in range(dd, 4):
                        last = (skt == nsk - 1 - (3 - qc))
                        nc.tensor.matmul(
                            den_ps[:, qc:qc + 1],
                            lhsT=exm[:, (qc - dd) * 128:(qc - dd + 1) * 128],
                            rhs=ones_sb,
                            start=(first and qc == 0),
                            stop=last,
                            skip_group_check=True)
                        if last:
                            nc.vector.reciprocal(rcb_c[:, qc:qc + 1],
                                                 den_ps[:, qc:qc + 1])
                            nc.gpsimd.dma_start(
                                out=rcb_row[:, qc * 128:(qc + 1) * 128],
                                in_=rcb_c[:, qc:qc + 1])
                # free the pv psum bank quickly: copy the unnormalised
                # accumulator to SBUF; the rest of the normalisation pipeline
                # runs off the PE critical path (its deadline is the deferred
                # wo of this strip, a full strip away).
                pv_sb = rp.tile([128, STRIP], BF16, tag="pvs")
                nc.vector.tensor_copy(pv_sb, pv_ps)
                rcb_bc = nrm.tile([128, STRIP], F32, tag="rcbb")
                nc.gpsimd.partition_broadcast(rcb_bc, rcb_row)
                m_sb = rp.tile([128, STRIP], BF16, tag="msb")
                nc.vector.tensor_mul(m_sb, pv_sb, rcb_bc)
                if j >= V_OT_DVE:
                    nc.vector.tensor_copy(otile[:, h, 0, :], m_sb)
                else:
                    nc.scalar.copy(otile[:, h, 0, :], m_sb)
                nc.vector.tensor_sub(otile[:, h, 1, :], m_sb, otile[:, h, 0, :])

            while j == NSTRIP - 1 and deferred:
                _emit_wo_tile(*deferred.pop(0))
            # wo for this strip is deferred into the next strip's attention
            # window (PE-light there); strip 3 emits inline above.
            if j < NSTRIP - 1:
                deferred.extend((j, otile, nt) for nt in range(NDT))
            else:
                for nt in range(NDT):
                    _emit_wo_tile(j, otile, nt, borrow=True)

    return nc


_PERM = np.concatenate([np.arange(0, HD, 2), np.arange(1, HD, 2)])


def _split_fp8(a):
    """Split f32 array into (hi, lo) fp8e4 planes with hi + lo ~ a."""
    fp8 = ml_dtypes.float8_e4m3
    hi = a.astype(fp8)
    lo = (a - hi.astype(np.float32)).astype(fp8)
    return hi, lo


def _host_prep(x, wq, wk, wv, wo, freqs_cos, freqs_sin, mask):
    bf16 = ml_dtypes.bfloat16
    x = np.asarray(x, np.float32)
    wq = np.asarray(wq, np.float32)
    wk = np.asarray(wk, np.float32)
    wv = np.asarray(wv, np.float32)
    wo = np.asarray(wo, np.float32)
    cos = np.asarray(freqs_cos, np.float32)   # [S, HD/2]
    sin = np.asarray(freqs_sin, np.float32)
    mask = np.asarray(mask, np.float32)

    cosH = cos.T                               # [64, S]
    sinH = sin.T
    cs = np.vstack([cosH, cosH]).astype(bf16)  # [128, S]
    sn = np.vstack([sinH, sinH]).astype(bf16)

    # multiplicative binary causal mask for the 4 diagonal-tile flavours:
    # mk[k, d, q] = 1 where allowed, 0 where masked
    mk = np.empty((SKT, 4, STRIP), np.float32)
    for d_ in range(4):
        sub = mask[0:STRIP, d_ * SKT:(d_ + 1) * SKT]   # [q, k]
        mk[:, d_, :] = np.where(np.isfinite(sub), 1.0, 0.0).T
    mk = mk.astype(bf16)

    perm_g = np.concatenate([h * HD + _PERM for h in range(HPG)])

    # lhsT of the rope pair-mix matmul: (J q) rows 0:64 = -q[64:128],
    # rows 64:128 = +q[0:64]; jt = J.T
    jt = np.zeros((HD, HD), np.float32)
    jt[np.arange(64), np.arange(64) + 64] = 1.0
    jt[np.arange(64) + 64, np.arange(64)] = -1.0
    jt = jt.astype(bf16)

    def _pack_w(wT):
        # wT: [D, EG] f32 -> [128, NDT, 2(lo,hi), EG] fp8, scaled by WSCALE
        hi, lo = _split_fp8(wT * WSCALE)
        st = np.stack([lo, hi], axis=1)            # [D, 2, EG]
        return np.ascontiguousarray(
            st.reshape(NDT, 128, 2, EG).transpose(1, 0, 2, 3))

    def _pack_x(xT):
        # xT: [D, S] f32 -> [128, NDT, 2(hi,lo), S] fp8
        hi, lo = _split_fp8(xT)
        st = np.stack([hi, lo], axis=1)            # [D, 2, S]
        return np.ascontiguousarray(
            st.reshape(NDT, 128, 2, S).transpose(1, 0, 2, 3))

    in_maps = []
    for c in range(NCORES):
        b, g = c // HPG, c % HPG
        rows = slice(g * EG, (g + 1) * EG)
        wq_g = wq[rows][perm_g]                # [EG, D], head dims permuted
        wk_g = wk[rows][perm_g]
        wv_g = wv[rows]
        wo_g = wo[:, rows]                     # [D, EG]
        # woT: [EG, D] -> [128, HPG(h-chunk), 2(lo,hi), D]
        hi, lo = _split_fp8(wo_g.T * WSCALE)
        wo_st = np.stack([lo, hi], axis=1)     # [EG, 2, D]
        wo_pk = np.ascontiguousarray(
            wo_st.reshape(HPG, 128, 2, D).transpose(1, 0, 2, 3))
        in_maps.append({
            "xT": _pack_x(x[b].T),
            "wqT": _pack_w(wq_g.T),
            "wkT": _pack_w(wk_g.T),
            "wvT": _pack_w(wv_g.T),
            "woT": wo_pk,
            "cs": cs, "sn": sn, "mk": mk, "jt": jt,
        })
    return in_maps


def kernel(x, wq, wk, wv, wo, freqs_cos, freqs_sin, mask, start_pos):
    global LAST_EXEC_NS, LAST_RESULTS
    in_maps = _host_prep(x, wq, wk, wv, wo, freqs_cos, freqs_sin, mask)
    nc = _build_program()
    nc.finalize()
    res = run_bass_kernel_spmd(nc, in_maps, core_ids=list(range(NCORES)),
                               trace=False)
    LAST_EXEC_NS = res.exec_time_ns
    LAST_RESULTS = res
    out = np.empty((B, S, D), np.float32)
    for b in range(B):
        acc = np.zeros((D, S), np.float32)
        for g in range(HPG):
            acc += res.results[b * HPG + g]["outT"].astype(np.float32)
        out[b] = acc.T
    return out


# revision 33
# speedup vs baseline: 1.1398x; 1.0027x over previous
"""Self-contained Trainium2 Bass kernel for multi-head causal attention with RoPE.

Problem: B=2, S=2048, D=2048, H=16 heads (HD=128), fp32 reference:
    q = rope(x @ wq.T), k = rope(x @ wk.T), v = x @ wv.T
    out = softmax(q k^T / sqrt(HD) + causal_mask) @ v @ wo.T

Sharding (8 cores): core c = (b, g) with b = c // 4 (batch), g = c % 4
(head-group of 4 heads).  Each core computes its head-group's attention for
its batch and a partial output projection (columns 512g:512g+512 of the
attention output times the matching wo rows).  The host sums the 4 partial
[D, S] tensors per batch and transposes back to [S, D].

Matmul precision: the four big projections (q, k, v, wo) run as fp8e4
DoubleRow matmuls with a 3-term hi/lo split (w*x ~ wh*xh + wl*xh + wh*xl).
DoubleRow contracts two 128-partition planes per pass at half the cycle
cost, so the 3-term split costs 0.75x the bf16 rows while keeping ~11 bits
of mantissa.  x and the weights are split host-side (hi/lo planes are
interleaved in the free dim: x planes ordered (hi, lo), weight planes
(lo, hi) so one cross-term instruction pairs (wl*xh, wh*xl)).  The
attention matmuls (scores, PV) stay bf16 (contraction 128 gains nothing
from DoubleRow).

Softmax denominators come from the PE: each masked-exp tile is used as a
stationary operand against a moving ones-vector, accumulating per-q-chunk
column sums [128q, 1] in a dedicated PSUM bank (cost ~1 row per tile).
The [128, 4] reciprocal is transposed to a [1, 512] row by four tiny
SWDGE (Pool-issued) DMAs and partition-broadcast on GpSimd, then a single
DVE multiply normalizes the PV accumulator.

On-chip layout is "transposed" as in the bf16 version: Q^T/K^T as
[head_dim, seq], scoresT[k, q] feeds PV directly.  RoPE pair mixing is a
PE matmul against a signed permutation J (head dims of wq/wk permuted
host-side).  Causal masking: strictly-above-diagonal tiles skipped,
diagonal tiles at partial width with a multiplicative binary bf16 mask
after exp.
"""

import math

import numpy as np
import ml_dtypes

import concourse.bass as bass
import concourse.bacc as bacc
import concourse.mybir as mybir
from concourse.tile import TileContext
from concourse.bass_utils import run_bass_kernel_spmd
from contextlib import ExitStack

B, S, D, H = 2, 2048, 2048, 16
HD = 128          # head dim
HPG = 4           # heads per core (group)
EG = HPG * HD     # 512 head dims per core
NCORES = 8
NSTRIP = 4        # q strips per sequence
STRIP = S // NSTRIP   # 512
SKT = 128         # k tile (partition dim of scoresT)
NDT = D // 128    # 16 contraction tiles for projections
SCALE = 1.0 / math.sqrt(HD)

BF16 = mybir.dt.bfloat16
F32 = mybir.dt.float32
FP8 = mybir.dt.float8e4
DR = mybir.MatmulPerfMode.DoubleRow

# Host-side weight scaling: the reference weights have sigma ~ 1/sqrt(D) =
# 0.022, whose fp8 lo-residual (~0.0014) would underflow e4m3's subnormal
# floor (2^-9).  Scaling every weight matrix by 2^5 puts both planes in the
# normal range; q*k picks up 2^10 (cancelled in the exp scale) and v*wo
# picks up 2^10 (cancelled in the output-projection copies).
WSCALE = 32.0

LAST_EXEC_NS = None
LAST_RESULTS = None

import os
V_PR_DVE = int(os.environ.get("V_PR_DVE", "1"))      # pr_sb copies: 0=alt, 1=all DVE, 2=all ACT
V_OT_DVE = int(os.environ.get("V_OT_DVE", "2"))      # otile-hi copy on DVE for j >= this (4=never)
V_QSB_DVE = int(os.environ.get("V_QSB_DVE", "1"))    # q_sb copy engine: 0=ACT, 1=DVE
V_KSB_ACT = int(os.environ.get("V_KSB_ACT", "0"))    # k_sb copy engine: 0=DVE, 1=ACT
V_EP_BUFS = int(os.environ.get("V_EP_BUFS", "6"))
V_PO_BUFS = int(os.environ.get("V_PO_BUFS", "6"))
V_WO_PER_HEAD = int(os.environ.get("V_WO_PER_HEAD", "3"))
V_TAIL_ALT = int(os.environ.get("V_TAIL_ALT", "1"))  # borrow-mode pr copies alternate DVE/ACT
V_START_HI = int(os.environ.get("V_START_HI", "0"))  # split first wq/xt DMA into hi-first
V_HOIST = int(os.environ.get("V_HOIST", "0"))        # high-priority projection matmuls for j>=1
V_MASKMM = int(os.environ.get("V_MASKMM", "0"))      # causal mask via PE accumulate instead of DVE mul
V_RCB2 = int(os.environ.get("V_RCB2", "0"))          # rcb transpose via 2-column DMAs
V_PVS_ACT = int(os.environ.get("V_PVS_ACT", "0"))    # pv_sb copy on ACT


def _build_program():
    nc = bacc.Bacc("TRN2", target_bir_lowering=False, debug=False,
                   num_devices=NCORES)
    # x / weights are pre-split into fp8 hi+lo planes host-side.
    # x planes: dim2 = (hi, lo); weight planes: dim2 = (lo, hi).
    xT_d = nc.dram_tensor("xT", [128, NDT, 2, S], FP8, kind="ExternalInput").ap()
    wqT_d = nc.dram_tensor("wqT", [128, NDT, 2, EG], FP8, kind="ExternalInput").ap()
    wkT_d = nc.dram_tensor("wkT", [128, NDT, 2, EG], FP8, kind="ExternalInput").ap()
    wvT_d = nc.dram_tensor("wvT", [128, NDT, 2, EG], FP8, kind="ExternalInput").ap()
    woT_d = nc.dram_tensor("woT", [128, HPG, 2, D], FP8, kind="ExternalInput").ap()
    cs_d = nc.dram_tensor("cs", [HD, S], BF16, kind="ExternalInput").ap()
    sn_d = nc.dram_tensor("sn", [HD, S], BF16, kind="ExternalInput").ap()
    mk_d = nc.dram_tensor("mk", [SKT, 4, STRIP], BF16, kind="ExternalInput").ap()
    jt_d = nc.dram_tensor("jt", [HD, HD], BF16, kind="ExternalInput").ap()
    dg_d = nc.dram_tensor("dg", [SKT, SKT], BF16, kind="ExternalInput").ap()
    outT_d = nc.dram_tensor("outT", [D, S], BF16, kind="ExternalOutput").ap()

    EXP = mybir.ActivationFunctionType.Exp

    with TileContext(nc) as tc, ExitStack() as ctx:
        wpool = ctx.enter_context(tc.tile_pool(name="wpool", bufs=1))
        kv = ctx.enter_context(tc.tile_pool(name="kv", bufs=1))
        xs = ctx.enter_context(tc.tile_pool(name="xs", bufs=2))
        qs = ctx.enter_context(tc.tile_pool(name="qs", bufs=2))
        rp = ctx.enter_context(tc.tile_pool(name="rp", bufs=2))
        ep = ctx.enter_context(tc.tile_pool(name="ep", bufs=V_EP_BUFS))
        ot = ctx.enter_context(tc.tile_pool(name="ot", bufs=3))
        po = ctx.enter_context(tc.tile_pool(name="po", bufs=V_PO_BUFS))
        nrm = ctx.enter_context(tc.tile_pool(name="nrm", bufs=int(os.environ.get("V_NRM_BUFS", "2"))))
        scps = ctx.enter_context(tc.tile_pool(name="scps", bufs=2, space="PSUM"))
        prps = ctx.enter_context(tc.tile_pool(name="prps", bufs=2, space="PSUM"))
        pvps = ctx.enter_context(tc.tile_pool(name="pvps", bufs=2, space="PSUM"))
        wops = ctx.enter_context(tc.tile_pool(name="wops", bufs=1, space="PSUM"))
        dnps = ctx.enter_context(tc.tile_pool(name="dnps", bufs=1, space="PSUM"))

        # persistent SBUF tensors
        wq_sb = wpool.tile([128, NDT, 2, EG], FP8)
        wk_sb = wpool.tile([128, NDT, 2, EG], FP8)
        wv_sb = wpool.tile([128, NDT, 2, EG], FP8)
        wo_sb = wpool.tile([128, HPG, 2, D], FP8)
        cs_sb = wpool.tile([128, S], BF16)
        sn_sb = wpool.tile([128, S], BF16)
        mk_sb = wpool.tile([128, 4, STRIP], BF16)
        jt_sb = wpool.tile([HD, HD], BF16)
        dg_sb = wpool.tile([SKT, SKT], BF16)
        ones_sb = wpool.tile([128, 1], BF16)
        KT_sb = kv.tile([128, HPG, S], BF16)       # [e, h, sk] rope'd K^T
        V_sb = kv.tile([128, S // 128, EG], BF16)  # [sk, sk_tile, e]

        nc.vector.memset(ones_sb, 1.0)

        xt0 = xs.tile([128, NDT, 2, STRIP], FP8, tag="xt")
        x0r = xT_d[:, :, :, 0:STRIP]

        # --- DMA issue order tuned for the critical path: the first q
        # projection needs wq chunk-pairs + the first x chunk-pairs.
        if V_START_HI:
            # hi planes of the first chunk-pair land first so the first
            # hi-hi matmul starts as early as possible
            nc.sync.dma_start(out=wq_sb[:, 0:2, 1], in_=wqT_d[:, 0:2, 1])
            nc.sync.dma_start(out=xt0[:, 0:2, 0], in_=x0r[:, 0:2, 0])
            nc.sync.dma_start(out=wq_sb[:, 0:2, 0], in_=wqT_d[:, 0:2, 0])
            nc.sync.dma_start(out=xt0[:, 0:2, 1], in_=x0r[:, 0:2, 1])
            pairs = ((2, 4), (4, 8), (8, 12), (12, 16))
        else:
            pairs = ((0, 2), (2, 4), (4, 8), (8, 12), (12, 16))
        for a, b in pairs:
            nc.sync.dma_start(out=wq_sb[:, a:b], in_=wqT_d[:, a:b])
            nc.sync.dma_start(out=xt0[:, a:b], in_=x0r[:, a:b])
        for c0 in range(0, NDT, 4):
            nc.sync.dma_start(out=wk_sb[:, c0:c0 + 4], in_=wkT_d[:, c0:c0 + 4])
        nc.sync.dma_start(out=jt_sb, in_=jt_d)
        nc.sync.dma_start(out=dg_sb, in_=dg_d)
        nc.sync.dma_start(out=wv_sb[:, 0:4], in_=wvT_d[:, 0:4])
        nc.sync.dma_start(out=cs_sb, in_=cs_d)
        nc.sync.dma_start(out=sn_sb, in_=sn_d)
        for c0 in range(4, NDT, 4):
            nc.sync.dma_start(out=wv_sb[:, c0:c0 + 4], in_=wvT_d[:, c0:c0 + 4])
        nc.sync.dma_start(out=mk_sb, in_=mk_d)
        xt1 = xs.tile([128, NDT, 2, STRIP], FP8, tag="xt")
        nc.sync.dma_start(out=xt1, in_=xT_d[:, :, :, STRIP:2 * STRIP])
        nc.sync.dma_start(out=wo_sb, in_=woT_d)

        def _proj3(out_ps, w_sb, xt, e0, ew, s0=0, sw=STRIP):
            """3-term fp8 DoubleRow accumulation: out_ps [128(e), sw] +=
            w[:, :, :, e0:e0+ew].T @ x[:, :, :, s0:s0+sw] over all 16 chunks.
            Weight planes are (lo, hi); x planes are (hi, lo)."""
            ni = NDT // 2 + NDT
            k = 0
            for t in range(NDT // 2):
                nc.tensor.matmul(out_ps,
                                 lhsT=w_sb[:, 2*t:2*t+2, 1, e0:e0+ew],
                                 rhs=xt[:, 2*t:2*t+2, 0, s0:s0+sw],
                                 start=(k == 0), stop=(k == ni - 1),
                                 perf_mode=DR)
                k += 1
            for c in range(NDT):
                nc.tensor.matmul(out_ps,
                                 lhsT=w_sb[:, c, :, e0:e0+ew],
                                 rhs=xt[:, c, :, s0:s0+sw],
                                 start=(k == 0), stop=(k == ni - 1),
                                 perf_mode=DR)
                k += 1

        def _projv3(out_ps, xt, st):
            """V projection: stationary = x chunk [128, 2, 128(s)],
            moving = wv planes [128, 2, 512(e)]."""
            ni = NDT // 2 + NDT
            k = 0
            sl = slice(st * 128, (st + 1) * 128)
            for t in range(NDT // 2):
                nc.tensor.matmul(out_ps,
                                 lhsT=xt[:, 2*t:2*t+2, 0, sl],
                                 rhs=wv_sb[:, 2*t:2*t+2, 1, :],
                                 start=(k == 0), stop=(k == ni - 1),
                                 perf_mode=DR)
                k += 1
            for c in range(NDT):
                nc.tensor.matmul(out_ps,
                                 lhsT=xt[:, c, :, sl],
                                 rhs=wv_sb[:, c, :, :],
                                 start=(k == 0), stop=(k == ni - 1),
                                 perf_mode=DR)
                k += 1

        deferred = []
        COPY = mybir.ActivationFunctionType.Copy
        UNSCALE = 1.0 / (WSCALE * WSCALE)

        def _emit_wo_tile(j, otile, nt, borrow=False):
            """One partial-output-projection tile: 3-term fp8 DR over the
            4 head chunks.  otile: [128(e), HPG, 2(hi,lo), STRIP] fp8.
            The copies also undo the host-side 2^10 weight scaling."""
            s0 = j * STRIP
            nsl = slice(nt * 128, (nt + 1) * 128)
            if borrow:
                # attention is over: rotate across every psum pool so the
                # tail pipelines instead of serialising on one bank
                r = nt % 4
                if r == 0:
                    pr = wops.tile([128, STRIP], F32, tag="pr")
                elif r == 1:
                    pr = scps.tile([128, STRIP], F32, tag="sc")
                elif r == 2:
                    pr = prps.tile([128, STRIP], F32, tag="mm")
                else:
                    pr = pvps.tile([128, STRIP], F32, tag="pv")
            else:
                pr = wops.tile([128, STRIP], F32, tag="pr")
            k = 0
            for t in range(HPG // 2):
                nc.tensor.matmul(pr,
                                 lhsT=wo_sb[:, 2*t:2*t+2, 1, nsl],
                                 rhs=otile[:, 2*t:2*t+2, 0, :],
                                 start=(k == 0), stop=False,
                                 perf_mode=DR)
                k += 1
            for h in range(HPG):
                nc.tensor.matmul(pr,
                                 lhsT=wo_sb[:, h, :, nsl],
                                 rhs=otile[:, h, :, :],
                                 start=False, stop=(h == HPG - 1),
                                 perf_mode=DR)
            pr_sb = po.tile([128, STRIP], BF16, tag="po")
            if borrow and V_TAIL_ALT:
                use_act = (nt % 2 == 1)
            else:
                use_act = (nt % 2 == 1) if V_PR_DVE == 0 else (V_PR_DVE == 2)
            if use_act:
                nc.scalar.activation(pr_sb, pr, COPY, scale=UNSCALE)
            else:
                nc.vector.tensor_scalar_mul(pr_sb, pr, UNSCALE)
            nc.sync.dma_start(
                out=outT_d[nt * 128:(nt + 1) * 128, s0:s0 + STRIP], in_=pr_sb)

        V_PIPE = int(os.environ.get("V_PIPE", "0"))

        strip_state = {}

        def emit_proj_head(j, h, xt, qt, pipelined):
            """Q/K projections + lag-1 J/rope for one head of strip j."""
            s0 = j * STRIP
            e0 = h * HD
            qk_sb = strip_state[j]["qk"]

            def _jrope(hh):
                for which, dst in (("q", qt[:, hh, :]),
                                   ("k", KT_sb[:, hh, s0:s0 + STRIP])):
                    src = qk_sb.pop((hh, which))
                    if j == 0:
                        jps = pvps.tile([128, STRIP], F32, tag="pv")
                    elif pipelined and os.environ.get("V_JPS_SC") == "1":
                        jps = scps.tile([128, STRIP], F32, tag="sc")
                    elif pipelined:
                        jps = prps.tile([128, STRIP], F32, tag="mm")
                    else:
                        jps = scps.tile([128, STRIP], F32, tag="sc")
                    nc.tensor.matmul(jps, lhsT=jt_sb, rhs=src,
                                     start=True, stop=True)
                    j_sb = rp.tile([128, STRIP], BF16, tag="jsb")
                    nc.scalar.copy(j_sb, jps)
                    t1 = rp.tile([128, STRIP], BF16, tag="ra")
                    nc.vector.tensor_mul(t1, src, cs_sb[:, s0:s0 + STRIP])
                    t2 = rp.tile([128, STRIP], BF16, tag="rb")
                    nc.vector.tensor_mul(t2, j_sb, sn_sb[:, s0:s0 + STRIP])
                    nc.vector.tensor_add(dst, t1, t2)
            strip_state[j]["jrope"] = _jrope

            if j == 0:
                q_ps = strip_state[0]["qps0"][h]
            else:
                q_ps = prps.tile([128, STRIP], F32, tag="mm")
                _proj3(q_ps, wq_sb, xt, e0, HD)
            q_sb = rp.tile([128, STRIP], BF16, tag="qsb", bufs=4)
            if V_QSB_DVE:
                nc.vector.tensor_copy(q_sb, q_ps)
            else:
                nc.scalar.copy(q_sb, q_ps)
            qk_sb[(h, "q")] = q_sb

            k_ps = prps.tile([128, STRIP], F32, tag="mm")
            _proj3(k_ps, wk_sb, xt, e0, HD)
            k_sb = rp.tile([128, STRIP], BF16, tag="ksb", bufs=4)
            if V_KSB_ACT:
                nc.scalar.copy(k_sb, k_ps)
            else:
                nc.vector.tensor_copy(k_sb, k_ps)
            qk_sb[(h, "k")] = k_sb
            if h > 0:
                _jrope(h - 1)

        def emit_proj_v(j, xt):
            for st in range(4):
                v_ps = prps.tile([128, EG], F32, tag="mm")
                _projv3(v_ps, xt, st)
                nc.vector.tensor_copy(V_sb[:, j * 4 + st, :], v_ps)
                if st == 0:
                    strip_state[j]["jrope"](HPG - 1)

        def emit_attn_head(j, h, qt, otile):
            s0 = j * STRIP
            nsk = 4 * j + 4
            e0 = h * HD
            pv_ps = pvps.tile([128, STRIP], F32, tag="pv")
            den_ps = dnps.tile([128, 4], F32, tag="dn")
            rcb_c = nrm.tile([128, 4], F32, tag="rcbc")
            rcb_row = nrm.tile([1, STRIP], F32, tag="rcbr")
            for skt in range(nsk):
                d = skt - 4 * j   # >= 0 on the diagonal block
                w = STRIP - 128 * d if d > 0 else STRIP
                dd = max(d, 0)
                first = (skt == 0)
                sc_ps = scps.tile([128, STRIP], F32, tag="sc")
                if V_MASKMM:
                    nc.tensor.matmul(sc_ps[:, 0:w],
                                     lhsT=KT_sb[:, h, skt * 128:(skt + 1) * 128],
                                     rhs=qt[:, h, STRIP - w:STRIP],
                                     start=True, stop=(d < 0))
                    if d >= 0:
                        # additive causal mask on the PE: accumulate
                        # -2^21 * invmask so exp underflows to zero
                        nc.tensor.matmul(sc_ps[:, 0:w], lhsT=dg_sb,
                                         rhs=mk_sb[:, d, STRIP - w:STRIP],
                                         start=False, stop=True)
                    ex = ep.tile([128, STRIP], BF16, tag="ex")
                    nc.scalar.activation(ex[:, 0:w], sc_ps[:, 0:w], EXP,
                                         scale=SCALE / (WSCALE * WSCALE))
                    exm = ex
                else:
                    nc.tensor.matmul(sc_ps[:, 0:w],
                                     lhsT=KT_sb[:, h, skt * 128:(skt + 1) * 128],
                                     rhs=qt[:, h, STRIP - w:STRIP],
                                     start=True, stop=True)
                    ex = ep.tile([128, STRIP], BF16, tag="ex")
                    nc.scalar.activation(ex[:, 0:w], sc_ps[:, 0:w], EXP,
                                         scale=SCALE / (WSCALE * WSCALE))
                    if d >= 0:
                        exm = ep.tile([128, STRIP], BF16, tag="exm")
                        nc.vector.tensor_mul(exm[:, 0:w], ex[:, 0:w],
                                             mk_sb[:, d, STRIP - w:STRIP])
                    else:
                        exm = ex
                nc.tensor.matmul(pv_ps[:, STRIP - w:STRIP],
                                 lhsT=V_sb[:, skt, e0:e0 + HD],
                                 rhs=exm[:, 0:w], start=first,
                                 stop=(skt == nsk - 1))
                # denominator columns: exm chunk as stationary x ones.
                # NOTE: start=True zeroes the whole PSUM bank, so only the
                # very first den matmul of the head-strip starts the bank;
                # every column then accumulates into the zeroed bank.
                for qc in range(dd, 4):
                    last = (skt == nsk - 1 - (3 - qc))
                    nc.tensor.matmul(
                        den_ps[:, qc:qc + 1],
                        lhsT=exm[:, (qc - dd) * 128:(qc - dd + 1) * 128],
                        rhs=ones_sb,
                        start=(first and qc == 0),
                        stop=last,
                        skip_group_check=True)
                    if last and not V_RCB2:
                        nc.vector.reciprocal(rcb_c[:, qc:qc + 1],
                                             den_ps[:, qc:qc + 1])
                        nc.gpsimd.dma_start(
                            out=rcb_row[:, qc * 128:(qc + 1) * 128],
                            in_=rcb_c[:, qc:qc + 1])
                    elif last and V_RCB2 and qc % 2 == 1:
                        nc.vector.reciprocal(rcb_c[:, qc - 1:qc + 1],
                                             den_ps[:, qc - 1:qc + 1])
                        nc.gpsimd.dma_start(
                            out=rcb_row[:, (qc - 1) * 128:(qc + 1) * 128]
                                .rearrange("p (c i) -> p i c", c=2),
                            in_=rcb_c[:, qc - 1:qc + 1])
            # free the pv psum bank quickly: copy the unnormalised
            # accumulator to SBUF; the rest of the normalisation pipeline
            # runs off the PE critical path.
            pv_sb = rp.tile([128, STRIP], BF16, tag="pvs")
            if V_PVS_ACT:
                nc.scalar.copy(pv_sb, pv_ps)
            else:
                nc.vector.tensor_copy(pv_sb, pv_ps)
            rcb_bc = nrm.tile([128, STRIP], F32, tag="rcbb")
            nc.gpsimd.partition_broadcast(rcb_bc, rcb_row)
            m_sb = rp.tile([128, STRIP], BF16, tag="msb")
            nc.vector.tensor_mul(m_sb, pv_sb, rcb_bc)
            if j >= V_OT_DVE:
                nc.vector.tensor_copy(otile[:, h, 0, :], m_sb)
            else:
                nc.scalar.copy(otile[:, h, 0, :], m_sb)
            nc.vector.tensor_sub(otile[:, h, 1, :], m_sb, otile[:, h, 0, :])

        xts = {0: xt0, 1: xt1}
        qts = {}
        for j in range(NSTRIP):
            s0 = j * STRIP
            strip_state[j] = {"qk": {}}
            if j == 0:
                # strip 0: chunk-outer over the 2-chunk DMA pairs so the first
                # matmuls start as soon as (wq pair 0, x pair 0) land; the
                # 4 concurrent head accumulators borrow the idle psum bufs
                xt = xts[0]
                qp0 = prps.tile([128, STRIP], F32, tag="mm")
                qp1 = prps.tile([128, STRIP], F32, tag="mm")
                qp2 = scps.tile([128, STRIP], F32, tag="sc")
                qp3 = scps.tile([128, STRIP], F32, tag="sc")
                qps0 = [qp0, qp1, qp2, qp3]
                strip_state[0]["qps0"] = qps0
                ni = NDT // 2 + NDT
                kk = [0] * HPG
                for t in range(NDT // 2):
                    for hh in range(HPG):
                        e0 = hh * HD
                        nc.tensor.matmul(qps0[hh],
                                         lhsT=wq_sb[:, 2*t:2*t+2, 1, e0:e0+HD],
                                         rhs=xt[:, 2*t:2*t+2, 0, :],
                                         start=(kk[hh] == 0), stop=False,
                                         perf_mode=DR)
                        kk[hh] += 1
                        for c in (2*t, 2*t+1):
                            nc.tensor.matmul(qps0[hh],
                                             lhsT=wq_sb[:, c, :, e0:e0+HD],
                                             rhs=xt[:, c, :, :],
                                             start=False, stop=(kk[hh] == ni - 1),
                                             perf_mode=DR)
                            kk[hh] += 1

            if j not in qts:
                qts[j] = qs.tile([128, HPG, STRIP], BF16, tag="qt", name=f"qt{j}")
            qt = qts[j]
            if not V_PIPE or j == 0:
                # non-pipelined: emit the whole projection phase up front
                for h in range(HPG):
                    emit_proj_head(j, h, xts[j], qt, pipelined=False)
                emit_proj_v(j, xts[j])

            # prefetch next strip's x and emit its qt tile
            if j + 1 < NSTRIP:
                if j + 1 not in xts:
                    xtn = xs.tile([128, NDT, 2, STRIP], FP8, tag="xt",
                                  name=f"xt{j+1}")
                    nc.sync.dma_start(
                        out=xtn,
                        in_=xT_d[:, :, :, (j + 1) * STRIP:(j + 2) * STRIP])
                    xts[j + 1] = xtn
                qts[j + 1] = qs.tile([128, HPG, STRIP], BF16, tag="qt",
                                     name=f"qt{j+1}")
                strip_state[j + 1] = {"qk": {}}

            # --- attention for this strip (+ pipelined next-strip work) ---
            otile = ot.tile([128, HPG, 2, STRIP], FP8, tag="ot")
            for h in range(HPG):
                # previous strip's deferred wo tiles, per head boundary
                nwo = V_WO_PER_HEAD
                for _ in range(nwo):
                    if deferred:
                        _emit_wo_tile(*deferred.pop(0))
                if V_PIPE == 1 and j + 1 < NSTRIP:
                    emit_proj_head(j + 1, h, xts[j + 1], qts[j + 1],
                                   pipelined=True)
                emit_attn_head(j, h, qt, otile)
                if V_PIPE == 2 and j + 1 < NSTRIP:
                    emit_proj_head(j + 1, h, xts[j + 1], qts[j + 1],
                                   pipelined=True)
            if V_PIPE and j + 1 < NSTRIP:
                emit_proj_v(j + 1, xts[j + 1])

            while j == NSTRIP - 1 and deferred:
                _emit_wo_tile(*deferred.pop(0))
            # wo for this strip is deferred into the next strip's attention
            # window (PE-light there); strip 3 emits inline above.
            if j < NSTRIP - 1:
                deferred.extend((j, otile, nt) for nt in range(NDT))
            else:
                for nt in range(NDT):
                    _emit_wo_tile(j, otile, nt, borrow=True)

    return nc


_PERM = np.concatenate([np.arange(0, HD, 2), np.arange(1, HD, 2)])


def _split_fp8(a):
    """Split f32 array into (hi, lo) fp8e4 planes with hi + lo ~ a."""
    fp8 = ml_dtypes.float8_e4m3
    hi = a.astype(fp8)
    lo = (a - hi.astype(np.float32)).astype(fp8)
    return hi, lo


def _host_prep(x, wq, wk, wv, wo, freqs_cos, freqs_sin, mask):
    bf16 = ml_dtypes.bfloat16
    x = np.asarray(x, np.float32)
    wq = np.asarray(wq, np.float32)
    wk = np.asarray(wk, np.float32)
    wv = np.asarray(wv, np.float32)
    wo = np.asarray(wo, np.float32)
    cos = np.asarray(freqs_cos, np.float32)   # [S, HD/2]
    sin = np.asarray(freqs_sin, np.float32)
    mask = np.asarray(mask, np.float32)

    cosH = cos.T                               # [64, S]
    sinH = sin.T
    cs = np.vstack([cosH, cosH]).astype(bf16)  # [128, S]
    sn = np.vstack([sinH, sinH]).astype(bf16)

    import os as _os
    maskmm = int(_os.environ.get("V_MASKMM", "0"))
    # causal mask for the 4 diagonal-tile flavours, [k, d, q]:
    # multiplicative (1 = allowed) by default, or inverse (1 = masked) when
    # the mask is applied as a -2^21 additive matmul on the PE
    mk = np.empty((SKT, 4, STRIP), np.float32)
    for d_ in range(4):
        sub = mask[0:STRIP, d_ * SKT:(d_ + 1) * SKT]   # [q, k]
        if maskmm:
            mk[:, d_, :] = np.where(np.isfinite(sub), 0.0, 1.0).T
        else:
            mk[:, d_, :] = np.where(np.isfinite(sub), 1.0, 0.0).T
    mk = mk.astype(bf16)
    dg = (-np.float32(2 ** 21) * np.eye(SKT, dtype=np.float32)).astype(bf16)

    perm_g = np.concatenate([h * HD + _PERM for h in range(HPG)])

    # lhsT of the rope pair-mix matmul: (J q) rows 0:64 = -q[64:128],
    # rows 64:128 = +q[0:64]; jt = J.T
    jt = np.zeros((HD, HD), np.float32)
    jt[np.arange(64), np.arange(64) + 64] = 1.0
    jt[np.arange(64) + 64, np.arange(64)] = -1.0
    jt = jt.astype(bf16)

    def _pack_w(wT):
        # wT: [D, EG] f32 -> [128, NDT, 2(lo,hi), EG] fp8, scaled by WSCALE
        hi, lo = _split_fp8(wT * WSCALE)
        st = np.stack([lo, hi], axis=1)            # [D, 2, EG]
        return np.ascontiguousarray(
            st.reshape(NDT, 128, 2, EG).transpose(1, 0, 2, 3))

    def _pack_x(xT):
        # xT: [D, S] f32 -> [128, NDT, 2(hi,lo), S] fp8
        hi, lo = _split_fp8(xT)
        st = np.stack([hi, lo], axis=1)            # [D, 2, S]
        return np.ascontiguousarray(
            st.reshape(NDT, 128, 2, S).transpose(1, 0, 2, 3))

    in_maps = []
    for c in range(NCORES):
        b, g = c // HPG, c % HPG
        rows = slice(g * EG, (g + 1) * EG)
        wq_g = wq[rows][perm_g]                # [EG, D], head dims permuted
        wk_g = wk[rows][perm_g]
        wv_g = wv[rows]
        wo_g = wo[:, rows]                     # [D, EG]
        # woT: [EG, D] -> [128, HPG(h-chunk), 2(lo,hi), D]
        hi, lo = _split_fp8(wo_g.T * WSCALE)
        wo_st = np.stack([lo, hi], axis=1)     # [EG, 2, D]
        wo_pk = np.ascontiguousarray(
            wo_st.reshape(HPG, 128, 2, D).transpose(1, 0, 2, 3))
        in_maps.append({
            "xT": _pack_x(x[b].T),
            "wqT": _pack_w(wq_g.T),
            "wkT": _pack_w(wk_g.T),
            "wvT": _pack_w(wv_g.T),
            "woT": wo_pk,
            "cs": cs, "sn": sn, "mk": mk, "jt": jt, "dg": dg,
        })
    return in_maps


def kernel(x, wq, wk, wv, wo, freqs_cos, freqs_sin, mask, start_pos):
    global LAST_EXEC_NS, LAST_RESULTS
    in_maps = _host_prep(x, wq, wk, wv, wo, freqs_cos, freqs_sin, mask)
    nc = _build_program()
    nc.finalize()
    res = run_bass_kernel_spmd(nc, in_maps, core_ids=list(range(NCORES)),
                               trace=False)
    LAST_EXEC_NS = res.exec_time_ns
    LAST_RESULTS = res
    out = np.empty((B, S, D), np.float32)
    for b in range(B):
        acc = np.zeros((D, S), np.float32)
        for g in range(HPG):
            acc += res.results[b * HPG + g]["outT"].astype(np.float32)
        out[b] = acc.T
    return out
